# revision 13
# baseline (speedup 1.0000x reference)
"""Trainium2 Bass kernel for nn_DecoderBlock (Autoformer decoder layer).

Data-parallel over batch: 16 batches -> 8 cores x 2 batches, per-batch
serial pipeline on each core. Layout is channel-major (E on partitions).

Autocorrelation without FFTs:
  M[tau] = (1/E) sum_t <k_t, q_{(t+tau)%L}>
via 128-row tiles of K^T Q whose wrapped diagonals are summed by a DRAM
shear round-trip (row stride 1153 on readback) + ones-matmul reduction.
Top-6 + softmax give a scattered weight vector; the roll-aggregation
  agg[e,t] = sum_{s'} Vrev[s',e] * c2R[s'+t]
uses host-time-reversed V inputs and the reversed weight vector written
twice to DRAM, so all access patterns are positive-stride.

Precision: score matmuls in true fp32 (top-6 margins ~1e-4 in M units);
FFN in bf16 (its output is ~0.2x the residual scale, so bf16 noise
dilutes well below tolerance); everything else float32r.
"""
import functools
import numpy as np

NCORES = 8
BPC = 2
L = 1024
E = 512
XP = 2048
F = 512
KS = 25
NCH = 4
NT = 8
NXP = 16
SCORES_F32R = True


@functools.lru_cache(maxsize=1)
def _build():
    import concourse.bacc as bacc
    import concourse.bass as bass
    import concourse.mybir as mybir
    from concourse.tile import TileContext

    F32 = mybir.dt.float32
    F32R = mybir.dt.float32r
    BF16 = mybir.dt.bfloat16
    AF = mybir.ActivationFunctionType
    OP = mybir.AluOpType

    SCORE_DT = F32R if SCORES_F32R else F32
    nc = bacc.Bacc("TRN2", target_bir_lowering=False, debug=False, num_devices=NCORES)

    def din(name, shape, dtype=F32R):
        return nc.declare_dram_parameter(name, list(shape), dtype, isOutput=False)

    xt_in = din("xt", (BPC, E, L))
    xtr_in = din("xtr", (BPC, E, L))
    ent_in = din("ent", (BPC, E, L))
    entr_in = din("entr", (BPC, E, L))
    w_in = {}
    for p in ("sa", "ca"):
        for nme in ("wq", "wk", "wv", "wo"):
            w_in[f"{p}_{nme}"] = din(f"{p}_{nme}", (E, E))
    fcw_in = din("fcw", (128, 16384), BF16)   # packed fc1(c,2048)|fc2(xc,512), bf16
    wct_in = din("wct", (3 * E, F))
    bias_pc_in = din("bias_pc", (128, 52), F32)
    brow_in = din("brow", (1, 2 * E), F32R)
    ident_in = din("ident", (128, 128), F32)
    identr_in = din("identr", (128, 128), F32R)
    jmat_in = din("jmat", (128, 128), F32)
    ones_div_in = din("ones_div", (128, 1))
    ones_row_in = din("ones_row", (1, 128))
    rampl_in = din("rampl", (128, 16), F32)
    ones12_in = din("ones12", (128, 12), F32)

    seas_out = nc.declare_dram_parameter("seasonal", [BPC, L, E], F32, isOutput=True)

    trend_out = nc.declare_dram_parameter("trend", [BPC, L, F], F32, isOutput=True)

    a2d = {(b, p): nc.dram_tensor(f"a2d_{p}{b}", [L, 1152], F32R)
           for b in range(BPC) for p in ("sa", "ca")}
    c2rd = {(b, p): nc.dram_tensor(f"c2rd_{p}{b}", [2 * L], F32R)
            for b in range(BPC) for p in ("sa", "ca")}
    trd = {b: nc.dram_tensor(f"trdram{b}", [NCH, 128, L], F32) for b in range(BPC)}

    BQ = {"sa": 0, "ca": 12}
    BK = {"sa": 4, "ca": 16}
    BO = {"sa": 8, "ca": 20}
    FC2B, LNG, LNB, FC1B = 24, 28, 32, 36
    BVROW = {"sa": 0, "ca": E}

    with TileContext(nc) as tc:
        with (
            tc.tile_pool(name="consts", bufs=1) as cpool,
            tc.tile_pool(name="wst", bufs=4) as wstp,
            tc.tile_pool(name="acts", bufs=1) as apool,
            tc.tile_pool(name="str", bufs=1) as spool,
            tc.tile_pool(name="pp", bufs=2, space="PSUM") as pspool,
            tc.tile_pool(name="pa", bufs=1, space="PSUM") as papool,
            tc.tile_pool(name="pm", bufs=1, space="PSUM") as pmpool,
        ):
            ident = cpool.tile([128, 128], F32, name="ident")
            identr = cpool.tile([128, 128], F32R, name="identr")
            jmat = cpool.tile([128, 128], F32, name="jmat")
            ones_div = cpool.tile([128, 1], F32R, name="ones_div")
            ones_row = cpool.tile([1, 128], F32R, name="ones_row")
            rampl = cpool.tile([128, 16], F32, name="rampl")
            ones12 = cpool.tile([128, 12], F32, name="ones12")
            bias_pc = cpool.tile([128, 52], F32, name="bias_pc")
            brow = cpool.tile([1, 2 * E], F32R, name="brow")
            for t, d in ((ident, ident_in), (identr, identr_in), (jmat, jmat_in),
                         (ones_div, ones_div_in), (ones_row, ones_row_in),
                         (rampl, rampl_in), (ones12, ones12_in),
                         (bias_pc, bias_pc_in), (brow, brow_in)):
                nc.sync.dma_start(out=t[:], in_=d.ap())

            def ps_tile(name):
                return pspool.tile([128, 512], F32, name=name, tag="pp")

            def wchunk(dram, sl0, sl1, name):
                """stream a (128, 512) f32r weight chunk: rows sl0 (c-range), cols sl1."""
                t = wstp.tile([128, 512], F32R, name=name, tag="wst")
                nc.sync.dma_start(
                    out=t[:].rearrange("p (c n) -> p c n", c=(sl0.stop - sl0.start))
                    if False else t[:],
                    in_=dram.ap().rearrange("(c p) n -> p c n", p=128)[:, sl0, sl1]
                    .rearrange("p c n -> p (c n)"))
                return t

            def proj_el(out_t, src_t, w_dram, bcol, resid=None, resid_dram=None):
                """out[e_out,t] = sum_e w[e,e_out] src[e,t] + bias (+resid)."""
                for m in range(NCH):
                    wm = wstp.tile([128, 512], F32R, name=f"wm{m}", tag="wst")
                    nc.sync.dma_start(
                        out=wm[:].rearrange("p (c n) -> p c n", c=NCH),
                        in_=w_dram.ap().rearrange("(c p) n -> p c n", p=128)
                            [:, :, 128 * m : 128 * m + 128])
                    for h in range(2):
                        pt = ps_tile(f"pp{m}{h}")
                        first = True
                        if resid is not None:
                            nc.tensor.matmul(pt[:], identr[:],
                                             resid[:, m * L + 512 * h : m * L + 512 * h + 512],
                                             start=True, stop=False)
                            first = False
                        elif resid_dram is not None:
                            rs = wstp.tile([128, 512], F32R, name=f"rs{m}{h}", tag="wst")
                            nc.sync.dma_start(
                                out=rs[:],
                                in_=resid_dram.rearrange("(c p) l -> p c l", p=128)
                                    [:, m, 512 * h : 512 * h + 512])
                            nc.tensor.matmul(pt[:], identr[:], rs[:], start=True, stop=False)
                            first = False
                        for c in range(NCH):
                            nc.tensor.matmul(
                                pt[:], wm[:, c * 128 : c * 128 + 128],
                                src_t[:, c * L + 512 * h : c * L + 512 * h + 512],
                                start=(first and c == 0), stop=(c == NCH - 1))
                        nc.vector.tensor_scalar(
                            out_t[:, m * L + 512 * h : m * L + 512 * h + 512], pt[:],
                            bias_pc[:, bcol + m : bcol + m + 1], None, op0=OP.add)

            def proj_rev(out_t, src_rev_t, w_dram, bvcol):
                """time-reversed V in (L,E): out[s',e] = sum_e' xrev[e',s'] w[e',e] + bv."""
                wv = []
                for c in range(NCH):
                    wc = wstp.tile([128, 512], F32R, name=f"wvc{c}", tag="wst")
                    nc.sync.dma_start(
                        out=wc[:],
                        in_=w_dram.ap().rearrange("(c p) n -> p c n", p=128)[:, c, :])
                    wv.append(wc)
                for j in range(NT):
                    pt = ps_tile(f"pv{j}")
                    nc.tensor.matmul(pt[:], ones_row[:], brow[0:1, bvcol : bvcol + E],
                                     start=True, stop=False)
                    for c in range(NCH):
                        nc.tensor.matmul(
                            pt[:], src_rev_t[:, c * L + 128 * j : c * L + 128 * j + 128],
                            wv[c][:], start=False, stop=(c == NCH - 1))
                    nc.vector.tensor_copy(out_t[:, j * E : (j + 1) * E], pt[:])

            def decomp(b, y_t, xnext_t, first_tr):
                """xnext = y - movavg25(y); DRAM trend (+)= movavg25(y)."""
                for c in range(NCH):
                    y = lambda a, bb: y_t[:, c * L + a : c * L + bb]
                    ws = spool.tile([128, L], F32, name=f"ws{c}", tag="ws", bufs=2)
                    cs25 = spool.tile([128, 25], F32, name=f"cs25{c}", tag="cs25", bufs=2)
                    nc.vector.tensor_tensor_scan(cs25[:], y(0, 25), y(0, 25), 0.0,
                                                 op0=OP.add, op1=OP.bypass)
                    nc.vector.scalar_tensor_tensor(ws[:, 0:13], rampl[:, 0:13], y(0, 1),
                                                   cs25[:, 12:25], op0=OP.mult, op1=OP.add)
                    nc.vector.tensor_tensor_scan(ws[:, 13:1012], y(25, L), y(0, 999),
                                                 cs25[:, 24:25], op0=OP.add, op1=OP.subtract)
                    ylast = spool.tile([128, 12], F32, name=f"yl{c}", tag="yl", bufs=2)
                    nc.vector.tensor_scalar(ylast[:], ones12[:], y(L - 1, L), None, op0=OP.mult)
                    nc.vector.tensor_tensor_scan(ws[:, 1012:1024], ylast[:], y(999, 1011),
                                                 ws[:, 1011:1012], op0=OP.add, op1=OP.subtract)
                    nc.vector.scalar_tensor_tensor(
                        xnext_t[:, c * L : (c + 1) * L], ws[:], -1.0 / KS, y(0, L),
                        op0=OP.mult, op1=OP.add)
                    # trend accumulation in DRAM: ws *= 1/25 then (accum-)DMA out
                    nc.vector.tensor_scalar(ws[:], ws[:], 1.0 / KS, None, op0=OP.mult)
                    if first_tr:
                        nc.gpsimd.dma_start(out=trd[b].ap()[c], in_=ws[:])
                    else:
                        nc.gpsimd.dma_start(out=trd[b].ap()[c], in_=ws[:],
                                            accum_op=OP.add)

            def attention(b, pfx, xq_t, kv_t, kvrev_t, out_y_t, resid=None, resid_dram=None):
                q_t = apool.tile([128, NCH * L], SCORE_DT, name=f"q{pfx}{b}", tag="q", bufs=1)
                k_t = apool.tile([128, NCH * L], SCORE_DT, name=f"k{pfx}{b}", tag="big32", bufs=1)
                proj_el(q_t, xq_t, w_in[f"{pfx}_wq"], BQ[pfx])
                proj_el(k_t, kv_t, w_in[f"{pfx}_wk"], BK[pfx])

                # ---- scores (fp32)
                a2 = a2d[(b, pfx)]
                m_ps = pmpool.tile([1, L], F32, name=f"mps{pfx}{b}", tag="psbig")
                for i in range(NT):
                    # pa banks: h=0 at 0, h=1 at 512, h=2 at 1024; the (at most
                    # one) wrapped split piece goes to its own bank at 1536 so
                    # every accumulation group has exactly one start=True owner.
                    pa = papool.tile([128, 2048], F32, name=f"pa{i}", tag="pa")
                    split_h, split_n1 = -1, 0
                    for c in range(NCH):
                        lhs = k_t[:, c * L + 128 * i : c * L + 128 * i + 128]
                        for h in range(3):
                            w0 = 128 * i + 384 * h
                            st, sp = (c == 0), (c == NCH - 1)
                            o = 512 * h
                            if w0 + 384 <= L:
                                nc.tensor.matmul(pa[:, o : o + 384], lhs,
                                                 q_t[:, c * L + w0 : c * L + w0 + 384],
                                                 start=st, stop=sp)
                            elif w0 >= L:
                                nc.tensor.matmul(pa[:, o : o + 384], lhs,
                                                 q_t[:, c * L + w0 - L : c * L + w0 - L + 384],
                                                 start=st, stop=sp)
                            else:
                                n1 = L - w0
                                split_h, split_n1 = h, n1
                                nc.tensor.matmul(pa[:, o : o + n1], lhs,
                                                 q_t[:, c * L + w0 : c * L + L],
                                                 start=st, stop=sp)
                                nc.tensor.matmul(pa[:, 1536 : 1536 + 384 - n1], lhs,
                                                 q_t[:, c * L : c * L + 384 - n1],
                                                 start=st, stop=sp)
                    a2sb = spool.tile([128, 1152], F32R, name=f"a2sb{i}", tag="a2sb", bufs=1)
                    for h in range(3):
                        if h == split_h:
                            n1 = split_n1
                            nc.vector.tensor_copy(a2sb[:, 384 * h : 384 * h + n1],
                                                  pa[:, 512 * h : 512 * h + n1])
                            nc.vector.tensor_copy(a2sb[:, 384 * h + n1 : 384 * h + 384],
                                                  pa[:, 1536 : 1536 + 384 - n1])
                        else:
                            nc.vector.tensor_copy(a2sb[:, 384 * h : 384 * h + 384],
                                                  pa[:, 512 * h : 512 * h + 384])
                    nc.sync.dma_start(out=a2.ap()[128 * i : 128 * i + 128, :], in_=a2sb[:])
                    sch = spool.tile([128, L], F32R, name=f"sch{i}", tag="sch", bufs=1)
                    nc.sync.dma_start(
                        out=sch[:], in_=bass.AP(a2, 1152 * 128 * i, [[1153, 128], [1, L]]))

                    for h in range(2):
                        nc.tensor.matmul(m_ps[0:1, 512 * h : 512 * h + 512], ones_div[:],
                                         sch[:, 512 * h : 512 * h + 512],
                                         start=(i == 0), stop=(i == NT - 1))

                # ---- V projection (after scores so "v" slot turns over late)
                vrev_t = apool.tile([128, NT * E], F32R, name=f"v{pfx}{b}", tag="v", bufs=1)
                proj_rev(vrev_t, kvrev_t, w_in[f"{pfx}_wv"], BVROW[pfx])

                # ---- reverse M; scattered softmax -> c2R
                m_row = spool.tile([1, L], F32, name=f"mrow{pfx}{b}", tag="mrow", bufs=1)
                nc.vector.tensor_copy(m_row[:], m_ps[:])
                colt_ps = pspool.tile([128, 8], F32, name=f"colt{pfx}{b}", tag="pp")
                for g in range(8):
                    nc.tensor.transpose(colt_ps[:, g : g + 1],
                                        m_row[0:1, 128 * g : 128 * g + 128], ident[0:1, 0:1])
                colt = spool.tile([128, 8], F32, name=f"coltsb{pfx}{b}", tag="coltsb", bufs=1)
                nc.vector.tensor_copy(colt[:], colt_ps[:])
                revt_ps = pspool.tile([128, 8], F32, name=f"revt{pfx}{b}", tag="pp")
                nc.tensor.matmul(revt_ps[:], jmat[:], colt[:], start=True, stop=True)
                revt = spool.tile([128, 8], F32, name=f"revtsb{pfx}{b}", tag="revtsb", bufs=1)
                nc.vector.tensor_copy(revt[:], revt_ps[:])
                mrev_ps = pmpool.tile([1, L], F32, name=f"mrevps{pfx}{b}", tag="psbig")
                for g in range(8):
                    nc.tensor.transpose(mrev_ps[0:1, 128 * (7 - g) : 128 * (7 - g) + 128],
                                        revt[:, g : g + 1], ident[:, :])
                mrev = spool.tile([1, L], F32, name=f"mrev{pfx}{b}", tag="mrev", bufs=1)
                nc.vector.tensor_copy(mrev[:], mrev_ps[:])

                top8 = spool.tile([1, 8], F32, name=f"top8{pfx}{b}", tag="top8", bufs=1)
                nc.vector.max(top8[:], mrev[:])
                negmax = spool.tile([1, 1], F32, name=f"ngm{pfx}{b}", tag="ngm", bufs=1)
                nc.vector.tensor_scalar(negmax[:], top8[:, 0:1], -1.0, None, op0=OP.mult)
                # mask overwrites m_row (dead); exp result then masked in place
                nc.vector.tensor_scalar(m_row[:], mrev[:], top8[:, 5:6], None, op0=OP.is_ge)
                nc.scalar.activation(mrev[:], mrev[:], AF.Exp, bias=negmax[:, 0:1])
                nc.vector.tensor_tensor(mrev[:], m_row[:], mrev[:], OP.mult)
                csum = spool.tile([1, 1], F32, name=f"csum{pfx}{b}", tag="csum", bufs=1)
                nc.vector.tensor_reduce(csum[:], mrev[:], axis=mybir.AxisListType.X, op=OP.add)
                crecip = spool.tile([1, 1], F32, name=f"crec{pfx}{b}", tag="crec", bufs=1)
                nc.vector.reciprocal(crecip[:], csum[:])
                c2r_sb = spool.tile([1, 2 * L], F32R, name=f"c2r{pfx}{b}", tag="c2r", bufs=1)
                nc.vector.tensor_scalar(c2r_sb[:, 0:L], mrev[:], crecip[:, 0:1], None, op0=OP.mult)
                nc.vector.tensor_scalar(c2r_sb[:, L:], mrev[:], crecip[:, 0:1], None, op0=OP.mult)
                nc.sync.dma_start(out=c2rd[(b, pfx)].ap(), in_=c2r_sb[:])

                # ---- aggregation
                cf = apool.tile([128, NT * L], F32R, name=f"cf{pfx}{b}", tag="big32", bufs=1)
                nc.sync.dma_start(
                    out=cf[:].rearrange("p (j l) -> p j l", j=NT),
                    in_=bass.AP(c2rd[(b, pfx)], 0, [[1, 128], [128, NT], [1, L]]))
                agg_t = apool.tile([128, NCH * L], F32R, name=f"agg{pfx}{b}", tag="agg", bufs=1)
                for m in range(NCH):
                    for h in range(2):
                        pt = ps_tile(f"pag{m}{h}")
                        for j in range(NT):
                            nc.tensor.matmul(
                                pt[:], vrev_t[:, j * E + 128 * m : j * E + 128 * m + 128],
                                cf[:, j * L + 512 * h : j * L + 512 * h + 512],
                                start=(j == 0), stop=(j == NT - 1))
                        nc.vector.tensor_copy(
                            agg_t[:, m * L + 512 * h : m * L + 512 * h + 512], pt[:])
                proj_el(out_y_t, agg_t, w_in[f"{pfx}_wo"], BO[pfx],
                        resid=resid, resid_dram=resid_dram)

            # ================= main program (per-batch serial) =================
            for b in range(BPC):
                xel = apool.tile([128, NCH * L], F32R, name=f"xel{b}", tag="xel", bufs=1)
                nc.sync.dma_start(out=xel[:].rearrange("p (c l) -> p c l", c=NCH),
                                  in_=xt_in.ap()[b].rearrange("(c p) l -> p c l", p=128))
                xrev = apool.tile([128, NCH * L], F32R, name=f"xrev{b}", tag="xrev", bufs=1)
                nc.sync.dma_start(out=xrev[:].rearrange("p (c l) -> p c l", c=NCH),
                                  in_=xtr_in.ap()[b].rearrange("(c p) l -> p c l", p=128))
                y1 = apool.tile([128, NCH * L], F32, name=f"y1{b}", tag="y", bufs=1)
                attention(b, "sa", xel, xel, xrev, y1, resid_dram=xt_in.ap()[b])
                x2 = apool.tile([128, NCH * L], F32R, name=f"x2{b}", tag="x", bufs=1)
                decomp(b, y1, x2, True)

                ent = apool.tile([128, NCH * L], F32R, name=f"ent{b}", tag="xel", bufs=1)
                nc.sync.dma_start(out=ent[:].rearrange("p (c l) -> p c l", c=NCH),
                                  in_=ent_in.ap()[b].rearrange("(c p) l -> p c l", p=128))
                entr = apool.tile([128, NCH * L], F32R, name=f"entr{b}", tag="xrev", bufs=1)
                nc.sync.dma_start(out=entr[:].rearrange("p (c l) -> p c l", c=NCH),
                                  in_=entr_in.ap()[b].rearrange("(c p) l -> p c l", p=128))
                y2 = apool.tile([128, NCH * L], F32, name=f"y2{b}", tag="y", bufs=1)
                attention(b, "ca", x2, ent, entr, y2, resid=x2)
                x3 = apool.tile([128, NCH * L], F32R, name=f"x3{b}", tag="x", bufs=1)
                decomp(b, y2, x3, False)

                # ---------------- FFN (bf16) ----------------
                fcw = apool.tile([128, 16384], BF16, name=f"fcw{b}", tag="big32", bufs=1)
                nc.sync.dma_start(out=fcw[:], in_=fcw_in.ap())
                x3bf = apool.tile([128, NCH * L], BF16, name=f"x3bf{b}", tag="xrev", bufs=1)
                nc.vector.tensor_copy(x3bf[:], x3[:])
                y3 = apool.tile([128, NCH * L], F32, name=f"y3{b}", tag="y", bufs=1)
                for half in range(2):
                    h_t = apool.tile([128, NXP * 512], BF16, name=f"h{b}{half}", tag="v", bufs=1)
                    for xc in range(NXP):
                        pt = ps_tile(f"ph{xc}")
                        for c in range(NCH):
                            nc.tensor.matmul(
                                pt[:], fcw[:, c * 2048 + 128 * xc : c * 2048 + 128 * xc + 128],
                                x3bf[:, c * L + 512 * half : c * L + 512 * half + 512],
                                start=(c == 0), stop=(c == NCH - 1))
                        nc.scalar.activation(h_t[:, xc * 512 : (xc + 1) * 512], pt[:],
                                             AF.Gelu, bias=bias_pc[:, FC1B + xc : FC1B + xc + 1])
                    for m in range(NCH):
                        pt = ps_tile(f"pf{m}")
                        for xc in range(NXP):
                            nc.tensor.matmul(
                                pt[:],
                                fcw[:, 8192 + xc * 512 + 128 * m : 8192 + xc * 512 + 128 * m + 128],
                                h_t[:, xc * 512 : (xc + 1) * 512],
                                start=(xc == 0), stop=(xc == NXP - 1))
                        sl = slice(m * L + 512 * half, m * L + 512 * half + 512)
                        nc.vector.scalar_tensor_tensor(
                            y3[:, sl], pt[:], bias_pc[:, FC2B + m : FC2B + m + 1],
                            x3[:, sl], op0=OP.add, op1=OP.add)
                x4 = apool.tile([128, NCH * L], F32R, name=f"x4{b}", tag="x", bufs=1)
                decomp(b, y3, x4, False)

                # ---------------- layernorm + seasonal ----------------
                sq = apool.tile([128, NCH * L], F32R, name=f"sq{b}", tag="agg", bufs=1)
                for c in range(NCH):
                    nc.scalar.activation(sq[:, c * L : (c + 1) * L],
                                         x4[:, c * L : (c + 1) * L], AF.Square)
                mu_ps = pmpool.tile([1, L], F32, name=f"mups{b}", tag="psbig")
                for h in range(2):
                    for c in range(NCH):
                        nc.tensor.matmul(mu_ps[0:1, 512 * h : 512 * h + 512], ones_div[:],
                                         x4[:, c * L + 512 * h : c * L + 512 * h + 512],
                                         start=(c == 0), stop=(c == NCH - 1))
                mu_r = spool.tile([1, L], F32, name=f"mur{b}", tag="mrow", bufs=1)
                nc.vector.tensor_copy(mu_r[:], mu_ps[:])
                ms_ps = pmpool.tile([1, L], F32, name=f"msps{b}", tag="psbig")
                for h in range(2):
                    for c in range(NCH):
                        nc.tensor.matmul(ms_ps[0:1, 512 * h : 512 * h + 512], ones_div[:],
                                         sq[:, c * L + 512 * h : c * L + 512 * h + 512],
                                         start=(c == 0), stop=(c == NCH - 1))
                var_r = spool.tile([1, L], F32, name=f"varr{b}", tag="mrev", bufs=1)
                nc.vector.tensor_tensor(var_r[:], mu_r[:], mu_r[:], OP.mult)
                nc.vector.scalar_tensor_tensor(var_r[:], ms_ps[:], 1e-5, var_r[:],
                                               op0=OP.add, op1=OP.subtract)
                nc.scalar.activation(var_r[:], var_r[:], AF.Sqrt)
                istd_r = spool.tile([1, L], F32, name=f"istdr{b}", tag="wexp", bufs=1)
                nc.vector.reciprocal(istd_r[:], var_r[:])
                rows = spool.tile([1, 2 * L], F32R, name=f"rows{b}", tag="c2r", bufs=1)
                nc.vector.tensor_copy(rows[:, 0:L], istd_r[:])
                nc.vector.tensor_tensor(rows[:, L:], mu_r[:], istd_r[:], OP.mult)
                bc = apool.tile([128, 2 * L], F32, name=f"bc{b}", tag="bc", bufs=1)
                for h in range(4):
                    bp = ps_tile(f"bc{h}")
                    nc.tensor.matmul(bp[:], ones_row[:], rows[0:1, 512 * h : 512 * h + 512],
                                     start=True, stop=True)
                    nc.vector.tensor_copy(bc[:, 512 * h : 512 * h + 512], bp[:])
                seas = apool.tile([128, NCH * L], F32, name=f"seas{b}", tag="q", bufs=1)
                accs = spool.tile([128, NCH], F32, name=f"accs{b}", tag="accs", bufs=1)
                for c in range(NCH):
                    t1 = spool.tile([128, L], F32, name=f"t1{c}", tag="ws", bufs=2)
                    nc.vector.tensor_tensor(t1[:], x4[:, c * L : (c + 1) * L],
                                            bc[:, 0:L], OP.mult)
                    nc.vector.tensor_tensor(t1[:], t1[:], bc[:, L:], OP.subtract)
                    nc.scalar.activation(seas[:, c * L : (c + 1) * L], t1[:], AF.Identity,
                                         bias=bias_pc[:, LNB + c : LNB + c + 1],
                                         scale=bias_pc[:, LNG + c : LNG + c + 1],
                                         accum_out=accs[:, c : c + 1])
                for c in range(NCH):
                    nc.vector.tensor_scalar(accs[:, c : c + 1], accs[:, c : c + 1],
                                            1.0 / L, None, op0=OP.mult)
                    nc.vector.tensor_scalar(seas[:, c * L : (c + 1) * L],
                                            seas[:, c * L : (c + 1) * L],
                                            accs[:, c : c + 1], None, op0=OP.subtract)
                for a in range(NT):
                    tp = ps_tile(f"tps{a}")
                    for c in range(NCH):
                        nc.tensor.transpose(tp[:, 128 * c : 128 * c + 128],
                                            seas[:, c * L + 128 * a : c * L + 128 * a + 128],
                                            ident[:, :])
                    osb = spool.tile([128, 512], F32, name=f"osb{a}", tag="osb", bufs=2)
                    nc.vector.tensor_copy(osb[:], tp[:])
                    nc.sync.dma_start(out=seas_out.ap()[b, 128 * a : 128 * a + 128, :],
                                      in_=osb[:])

                # ---------------- trend conv ----------------
                wct = apool.tile([128, 12 * F], F32R, name=f"wctt{b}", tag="big32", bufs=1)
                nc.sync.dma_start(out=wct[:].rearrange("p (c n) -> p c n", c=12),
                                  in_=wct_in.ap().rearrange("(c p) n -> p c n", p=128))
                tpad = apool.tile([128, NCH * 1026], F32R, name=f"tpad{b}", tag="y", bufs=1)
                nc.sync.dma_start(
                    out=tpad[:].rearrange("p (c l) -> p c l", c=NCH)[:, :, 1:1025],
                    in_=trd[b].ap().bitcast(F32R).rearrange("c p l -> p c l"))
                for c in range(NCH):
                    nc.vector.tensor_copy(tpad[:, c * 1026 : c * 1026 + 1],
                                          tpad[:, c * 1026 + L : c * 1026 + L + 1])
                    nc.vector.tensor_copy(tpad[:, c * 1026 + 1025 : c * 1026 + 1026],
                                          tpad[:, c * 1026 + 1 : c * 1026 + 2])
                for a in range(NT):
                    pt = ps_tile(f"ptc{a}")
                    n = 0
                    for j in range(3):
                        for c in range(NCH):
                            nc.tensor.matmul(
                                pt[:],
                                tpad[:, c * 1026 + 128 * a + j : c * 1026 + 128 * a + j + 128],
                                wct[:, (j * NCH + c) * F : (j * NCH + c) * F + F],
                                start=(n == 0), stop=(n == 11))
                            n += 1
                    osb = spool.tile([128, 512], F32, name=f"osc{a}", tag="osb", bufs=2)
                    nc.vector.tensor_copy(osb[:], pt[:])
                    nc.sync.dma_start(out=trend_out.ap()[b, 128 * a : 128 * a + 128, :],
                                      in_=osb[:])

    nc.compile()
    return nc


def _host_prep(inputs):
    f32 = np.float32
    x = np.asarray(inputs["x"], f32)
    enc = np.asarray(inputs["enc_output"], f32)
    xt = np.ascontiguousarray(x.transpose(0, 2, 1))
    xtr = np.ascontiguousarray(xt[:, :, ::-1])
    ent = np.ascontiguousarray(enc.transpose(0, 2, 1))
    entr = np.ascontiguousarray(ent[:, :, ::-1])

    shared = {}
    for p in ("sa", "ca"):
        for nme in ("wq", "wk", "wv", "wo"):
            shared[f"{p}_{nme}"] = np.ascontiguousarray(np.asarray(inputs[f"{p}_{nme}"], f32))
    import ml_dtypes
    fc1 = np.asarray(inputs["fc1_w"], f32).reshape(NCH, 128, XP)       # (c, p, xp)
    fc2 = np.asarray(inputs["fc2_w"], f32).reshape(NXP, 128, E)        # (xc, p, e)
    fcw = np.zeros((128, 16384), ml_dtypes.bfloat16)
    fcw[:, :8192] = fc1.transpose(1, 0, 2).reshape(128, 8192).astype(ml_dtypes.bfloat16)
    fcw[:, 8192:] = fc2.transpose(1, 0, 2).reshape(128, 8192).astype(ml_dtypes.bfloat16)
    shared["fcw"] = fcw
    tw = np.asarray(inputs["trend_w"], f32)
    shared["wct"] = np.ascontiguousarray(tw.transpose(2, 1, 0).reshape(3 * E, F))

    def pc(v, nch=4):
        return np.ascontiguousarray(np.asarray(v, f32).reshape(nch, 128).T)

    shared["bias_pc"] = np.ascontiguousarray(np.concatenate([
        pc(inputs["sa_bq"]), pc(inputs["sa_bk"]), pc(inputs["sa_bo"]),
        pc(inputs["ca_bq"]), pc(inputs["ca_bk"]), pc(inputs["ca_bo"]),
        pc(inputs["fc2_b"]), pc(inputs["ln_g"]), pc(inputs["ln_b"]),
        pc(inputs["fc1_b"], 16),
    ], axis=1))
    shared["brow"] = np.ascontiguousarray(np.concatenate(
        [np.asarray(inputs["sa_bv"], f32), np.asarray(inputs["ca_bv"], f32)])[None, :])
    shared["ident"] = np.eye(128, dtype=f32)
    shared["identr"] = np.eye(128, dtype=f32)
    shared["jmat"] = np.ascontiguousarray(np.eye(128, dtype=f32)[::-1])
    shared["ones_div"] = np.full((128, 1), 1.0 / E, f32)
    shared["ones_row"] = np.ones((1, 128), f32)
    ramp = np.zeros(16, f32)
    ramp[:13] = np.arange(12, -1, -1)
    shared["rampl"] = np.tile(ramp, (128, 1))
    shared["ones12"] = np.ones((128, 12), f32)

    in_maps = []
    for core in range(NCORES):
        s = slice(core * BPC, (core + 1) * BPC)
        m = dict(shared)
        m["xt"] = np.ascontiguousarray(xt[s])
        m["xtr"] = np.ascontiguousarray(xtr[s])
        m["ent"] = np.ascontiguousarray(ent[s])
        m["entr"] = np.ascontiguousarray(entr[s])
        in_maps.append(m)
    return in_maps


_LAST = {}


def kernel(**inputs):
    from concourse.bass_utils import run_bass_kernel_spmd

    nc = _build()
    in_maps = _host_prep(inputs)
    res = run_bass_kernel_spmd(nc, in_maps, core_ids=list(range(NCORES)),
                               **_LAST.get("kwargs", {}))
    _LAST["res"] = res
    seasonal = np.concatenate([res.results[c]["seasonal"] for c in range(NCORES)], axis=0)
    trend = np.concatenate([res.results[c]["trend"] for c in range(NCORES)], axis=0)
    return seasonal, trend


# revision 20
# speedup vs baseline: 1.1155x; 1.1155x over previous
"""Trainium2 Bass kernel for nn_DecoderBlock (Autoformer decoder layer).

Data-parallel over batch: 16 batches -> 8 cores x 2 batches, per-batch
serial pipeline on each core. Layout is channel-major (E on partitions).

Autocorrelation without FFTs:
  M[tau] = (1/E) sum_t <k_t, q_{(t+tau)%L}>
via 128-row tiles of K^T Q whose wrapped diagonals are summed by a DRAM
shear round-trip (row stride 1153 on readback) + ones-matmul reduction.
Top-6 + softmax give a scattered weight vector; the roll-aggregation
  agg[e,t] = sum_{s'} Vrev[s',e] * c2R[s'+t]
uses host-time-reversed V inputs and the reversed weight vector written
twice to DRAM, so all access patterns are positive-stride.

Precision: score matmuls in true fp32 (top-6 margins ~1e-4 in M units);
FFN in bf16 (its output is ~0.2x the residual scale, so bf16 noise
dilutes well below tolerance); everything else float32r.
"""
import functools
import numpy as np

NCORES = 8
BPC = 2
L = 1024
E = 512
XP = 2048
F = 512
KS = 25
NCH = 4
NT = 8
NXP = 16
SCORES_F32R = True


@functools.lru_cache(maxsize=1)
def _build():
    import concourse.bacc as bacc
    import concourse.bass as bass
    import concourse.mybir as mybir
    from concourse.tile import TileContext

    F32 = mybir.dt.float32
    F32R = mybir.dt.float32r
    BF16 = mybir.dt.bfloat16
    AF = mybir.ActivationFunctionType
    OP = mybir.AluOpType

    SCORE_DT = F32R if SCORES_F32R else F32
    nc = bacc.Bacc("TRN2", target_bir_lowering=False, debug=False, num_devices=NCORES)

    def din(name, shape, dtype=F32R):
        return nc.declare_dram_parameter(name, list(shape), dtype, isOutput=False)

    xt_in = din("xt", (BPC, E, L))
    xtr_in = din("xtr", (BPC, E, L))
    ent_in = din("ent", (BPC, E, L))
    entr_in = din("entr", (BPC, E, L))
    w_in = {}
    for p in ("sa", "ca"):
        for nme in ("wq", "wk", "wv", "wo"):
            w_in[f"{p}_{nme}"] = din(f"{p}_{nme}", (E, E))
    fcw_in = din("fcw", (128, 16384), BF16)   # packed fc1(c,2048)|fc2(xc,512), bf16
    wct_in = din("wct", (3 * E, F))
    bias_pc_in = din("bias_pc", (128, 52), F32)
    brow_in = din("brow", (1, 2 * E), F32R)
    ident_in = din("ident", (128, 128), F32)
    identr_in = din("identr", (128, 128), F32R)
    jmat_in = din("jmat", (128, 128), F32)
    ones_div_in = din("ones_div", (128, 1))
    ones_row_in = din("ones_row", (1, 128))
    rampl_in = din("rampl", (128, 16), F32)
    ones12_in = din("ones12", (128, 12), F32)

    seas_out = nc.declare_dram_parameter("seasonal", [BPC, L, E], F32, isOutput=True)

    trend_out = nc.declare_dram_parameter("trend", [BPC, L, F], F32, isOutput=True)

    a2d = {(b, p): nc.dram_tensor(f"a2d_{p}{b}", [L, 1152], F32R)
           for b in range(BPC) for p in ("sa", "ca")}
    c2rd = {(b, p): nc.dram_tensor(f"c2rd_{p}{b}", [2 * L], BF16)
            for b in range(BPC) for p in ("sa", "ca")}
    trd = {b: nc.dram_tensor(f"trdram{b}", [NCH, 128, L], F32) for b in range(BPC)}

    BQ = {"sa": 0, "ca": 12}
    BK = {"sa": 4, "ca": 16}
    BO = {"sa": 8, "ca": 20}
    FC2B, LNG, LNB, FC1B = 24, 28, 32, 36
    BVROW = {"sa": 0, "ca": E}

    with TileContext(nc) as tc:
        with (
            tc.tile_pool(name="consts", bufs=1) as cpool,
            tc.tile_pool(name="wst", bufs=4) as wstp,
            tc.tile_pool(name="acts", bufs=1) as apool,
            tc.tile_pool(name="str", bufs=1) as spool,
            tc.tile_pool(name="pp", bufs=2, space="PSUM") as pspool,
            tc.tile_pool(name="pa", bufs=1, space="PSUM") as papool,
            tc.tile_pool(name="pm", bufs=1, space="PSUM") as pmpool,
        ):
            ident = cpool.tile([128, 128], F32, name="ident")
            identr = cpool.tile([128, 128], F32R, name="identr")
            jmat = cpool.tile([128, 128], F32, name="jmat")
            ones_div = cpool.tile([128, 1], F32R, name="ones_div")
            ones_row = cpool.tile([1, 128], F32R, name="ones_row")
            rampl = cpool.tile([128, 16], F32, name="rampl")
            ones12 = cpool.tile([128, 12], F32, name="ones12")
            bias_pc = cpool.tile([128, 52], F32, name="bias_pc")
            brow = cpool.tile([1, 2 * E], F32R, name="brow")
            for t, d in ((ident, ident_in), (identr, identr_in), (jmat, jmat_in),
                         (ones_div, ones_div_in), (ones_row, ones_row_in),
                         (rampl, rampl_in), (ones12, ones12_in),
                         (bias_pc, bias_pc_in), (brow, brow_in)):
                nc.sync.dma_start(out=t[:], in_=d.ap())

            def ps_tile(name):
                return pspool.tile([128, 512], F32, name=name, tag="pp")

            def wchunk(dram, sl0, sl1, name):
                """stream a (128, 512) f32r weight chunk: rows sl0 (c-range), cols sl1."""
                t = wstp.tile([128, 512], F32R, name=name, tag="wst")
                nc.sync.dma_start(
                    out=t[:].rearrange("p (c n) -> p c n", c=(sl0.stop - sl0.start))
                    if False else t[:],
                    in_=dram.ap().rearrange("(c p) n -> p c n", p=128)[:, sl0, sl1]
                    .rearrange("p c n -> p (c n)"))
                return t

            def proj_el(out_t, src_t, w_dram, bcol, resid=None, resid_dram=None):
                """out[e_out,t] = sum_e w[e,e_out] src[e,t] + bias (+resid)."""
                for m in range(NCH):
                    wm = wstp.tile([128, 512], F32R, name=f"wm{m}", tag="wst")
                    nc.sync.dma_start(
                        out=wm[:].rearrange("p (c n) -> p c n", c=NCH),
                        in_=w_dram.ap().rearrange("(c p) n -> p c n", p=128)
                            [:, :, 128 * m : 128 * m + 128])
                    for h in range(2):
                        pt = ps_tile(f"pp{m}{h}")
                        first = True
                        if resid is not None:
                            nc.tensor.matmul(pt[:], identr[:],
                                             resid[:, m * L + 512 * h : m * L + 512 * h + 512],
                                             start=True, stop=False)
                            first = False
                        elif resid_dram is not None:
                            rs = wstp.tile([128, 512], F32R, name=f"rs{m}{h}", tag="wst")
                            nc.sync.dma_start(
                                out=rs[:],
                                in_=resid_dram.rearrange("(c p) l -> p c l", p=128)
                                    [:, m, 512 * h : 512 * h + 512])
                            nc.tensor.matmul(pt[:], identr[:], rs[:], start=True, stop=False)
                            first = False
                        for c in range(NCH):
                            nc.tensor.matmul(
                                pt[:], wm[:, c * 128 : c * 128 + 128],
                                src_t[:, c * L + 512 * h : c * L + 512 * h + 512],
                                start=(first and c == 0), stop=(c == NCH - 1))
                        nc.vector.tensor_scalar(
                            out_t[:, m * L + 512 * h : m * L + 512 * h + 512], pt[:],
                            bias_pc[:, bcol + m : bcol + m + 1], None, op0=OP.add)

            def proj_rev(out_t, src_rev_t, w_dram, bvcol):
                """time-reversed V in (L,E): out[s',e] = sum_e' xrev[e',s'] w[e',e] + bv."""
                wv = []
                for c in range(NCH):
                    wc = wstp.tile([128, 512], F32R, name=f"wvc{c}", tag="wst")
                    nc.sync.dma_start(
                        out=wc[:],
                        in_=w_dram.ap().rearrange("(c p) n -> p c n", p=128)[:, c, :])
                    wv.append(wc)
                for j in range(NT):
                    pt = ps_tile(f"pv{j}")
                    nc.tensor.matmul(pt[:], ones_row[:], brow[0:1, bvcol : bvcol + E],
                                     start=True, stop=False)
                    for c in range(NCH):
                        nc.tensor.matmul(
                            pt[:], src_rev_t[:, c * L + 128 * j : c * L + 128 * j + 128],
                            wv[c][:], start=False, stop=(c == NCH - 1))
                    nc.vector.tensor_copy(out_t[:, j * E : (j + 1) * E], pt[:])

            def decomp(b, y_t, xnext_t, first_tr):
                """xnext = y - movavg25(y); DRAM trend (+)= movavg25(y)."""
                for c in range(NCH):
                    y = lambda a, bb: y_t[:, c * L + a : c * L + bb]
                    ws = spool.tile([128, L], F32, name=f"ws{c}", tag="ws", bufs=2)
                    cs25 = spool.tile([128, 25], F32, name=f"cs25{c}", tag="cs25", bufs=1)
                    nc.vector.tensor_tensor_scan(cs25[:], y(0, 25), y(0, 25), 0.0,
                                                 op0=OP.add, op1=OP.bypass)
                    nc.vector.scalar_tensor_tensor(ws[:, 0:13], rampl[:, 0:13], y(0, 1),
                                                   cs25[:, 12:25], op0=OP.mult, op1=OP.add)
                    nc.vector.tensor_tensor_scan(ws[:, 13:1012], y(25, L), y(0, 999),
                                                 cs25[:, 24:25], op0=OP.add, op1=OP.subtract)
                    ylast = spool.tile([128, 12], F32, name=f"yl{c}", tag="yl", bufs=2)
                    nc.vector.tensor_scalar(ylast[:], ones12[:], y(L - 1, L), None, op0=OP.mult)
                    nc.vector.tensor_tensor_scan(ws[:, 1012:1024], ylast[:], y(999, 1011),
                                                 ws[:, 1011:1012], op0=OP.add, op1=OP.subtract)
                    nc.vector.scalar_tensor_tensor(
                        xnext_t[:, c * L : (c + 1) * L], ws[:], -1.0 / KS, y(0, L),
                        op0=OP.mult, op1=OP.add)
                    # trend accumulation in DRAM: ws *= 1/25 then (accum-)DMA out
                    nc.vector.tensor_scalar(ws[:], ws[:], 1.0 / KS, None, op0=OP.mult)
                    if first_tr:
                        nc.gpsimd.dma_start(out=trd[b].ap()[c], in_=ws[:])
                    else:
                        nc.gpsimd.dma_start(out=trd[b].ap()[c], in_=ws[:],
                                            accum_op=OP.add)

            def attention(b, pfx, xq_t, kv_t, kvrev_t, out_y_t, resid=None, resid_dram=None):
                q_t = apool.tile([128, NCH * L], SCORE_DT, name=f"q{pfx}{b}", tag="q", bufs=1)
                k_t = apool.tile([128, NCH * L], SCORE_DT, name=f"k{pfx}{b}", tag="big32", bufs=1)
                proj_el(q_t, xq_t, w_in[f"{pfx}_wq"], BQ[pfx])
                proj_el(k_t, kv_t, w_in[f"{pfx}_wk"], BK[pfx])

                # ---- scores (fp32)
                a2 = a2d[(b, pfx)]
                m_ps = pmpool.tile([1, L], F32, name=f"mps{pfx}{b}", tag="psbig")
                for i in range(NT):
                    # pa banks: h=0 at 0, h=1 at 512, h=2 at 1024; the (at most
                    # one) wrapped split piece goes to its own bank at 1536 so
                    # every accumulation group has exactly one start=True owner.
                    pa = papool.tile([128, 2048], F32, name=f"pa{i}", tag="pa")
                    split_h, split_n1 = -1, 0
                    for c in range(NCH):
                        lhs = k_t[:, c * L + 128 * i : c * L + 128 * i + 128]
                        for h in range(3):
                            w0 = 128 * i + 384 * h
                            st, sp = (c == 0), (c == NCH - 1)
                            o = 512 * h
                            if w0 + 384 <= L:
                                nc.tensor.matmul(pa[:, o : o + 384], lhs,
                                                 q_t[:, c * L + w0 : c * L + w0 + 384],
                                                 start=st, stop=sp)
                            elif w0 >= L:
                                nc.tensor.matmul(pa[:, o : o + 384], lhs,
                                                 q_t[:, c * L + w0 - L : c * L + w0 - L + 384],
                                                 start=st, stop=sp)
                            else:
                                n1 = L - w0
                                split_h, split_n1 = h, n1
                                nc.tensor.matmul(pa[:, o : o + n1], lhs,
                                                 q_t[:, c * L + w0 : c * L + L],
                                                 start=st, stop=sp)
                                nc.tensor.matmul(pa[:, 1536 : 1536 + 384 - n1], lhs,
                                                 q_t[:, c * L : c * L + 384 - n1],
                                                 start=st, stop=sp)
                    a2sb = spool.tile([128, 1152], F32R, name=f"a2sb{i}", tag="a2sb", bufs=1)
                    for h in range(3):
                        if h == split_h:
                            n1 = split_n1
                            nc.vector.tensor_copy(a2sb[:, 384 * h : 384 * h + n1],
                                                  pa[:, 512 * h : 512 * h + n1])
                            nc.vector.tensor_copy(a2sb[:, 384 * h + n1 : 384 * h + 384],
                                                  pa[:, 1536 : 1536 + 384 - n1])
                        else:
                            nc.vector.tensor_copy(a2sb[:, 384 * h : 384 * h + 384],
                                                  pa[:, 512 * h : 512 * h + 384])
                    nc.sync.dma_start(out=a2.ap()[128 * i : 128 * i + 128, :], in_=a2sb[:])
                    sch = spool.tile([128, L], F32R, name=f"sch{i}", tag="ws", bufs=2)
                    nc.sync.dma_start(
                        out=sch[:], in_=bass.AP(a2, 1152 * 128 * i, [[1153, 128], [1, L]]))

                    for h in range(2):
                        nc.tensor.matmul(m_ps[0:1, 512 * h : 512 * h + 512], ones_div[:],
                                         sch[:, 512 * h : 512 * h + 512],
                                         start=(i == 0), stop=(i == NT - 1))

                # ---- V projection (after scores so "v" slot turns over late)
                vrev_t = apool.tile([128, NT * E], BF16, name=f"v{pfx}{b}", tag="v", bufs=1)
                proj_rev(vrev_t, kvrev_t, w_in[f"{pfx}_wv"], BVROW[pfx])

                # ---- reverse M; scattered softmax -> c2R
                m_row = spool.tile([1, L], F32, name=f"mrow{pfx}{b}", tag="mrow", bufs=1)
                nc.vector.tensor_copy(m_row[:], m_ps[:])
                colt_ps = pspool.tile([128, 8], F32, name=f"colt{pfx}{b}", tag="pp")
                for g in range(8):
                    nc.tensor.transpose(colt_ps[:, g : g + 1],
                                        m_row[0:1, 128 * g : 128 * g + 128], ident[0:1, 0:1])
                colt = spool.tile([128, 8], F32, name=f"coltsb{pfx}{b}", tag="coltsb", bufs=1)
                nc.vector.tensor_copy(colt[:], colt_ps[:])
                revt_ps = pspool.tile([128, 8], F32, name=f"revt{pfx}{b}", tag="pp")
                nc.tensor.matmul(revt_ps[:], jmat[:], colt[:], start=True, stop=True)
                revt = spool.tile([128, 8], F32, name=f"revtsb{pfx}{b}", tag="revtsb", bufs=1)
                nc.vector.tensor_copy(revt[:], revt_ps[:])
                mrev_ps = pmpool.tile([1, L], F32, name=f"mrevps{pfx}{b}", tag="psbig")
                for g in range(8):
                    nc.tensor.transpose(mrev_ps[0:1, 128 * (7 - g) : 128 * (7 - g) + 128],
                                        revt[:, g : g + 1], ident[:, :])
                mrev = spool.tile([1, L], F32, name=f"mrev{pfx}{b}", tag="mrev", bufs=1)
                nc.vector.tensor_copy(mrev[:], mrev_ps[:])

                top8 = spool.tile([1, 8], F32, name=f"top8{pfx}{b}", tag="top8", bufs=1)
                nc.vector.max(top8[:], mrev[:])
                negmax = spool.tile([1, 1], F32, name=f"ngm{pfx}{b}", tag="ngm", bufs=1)
                nc.vector.tensor_scalar(negmax[:], top8[:, 0:1], -1.0, None, op0=OP.mult)
                # mask overwrites m_row (dead); exp result then masked in place
                nc.vector.tensor_scalar(m_row[:], mrev[:], top8[:, 5:6], None, op0=OP.is_ge)
                nc.scalar.activation(mrev[:], mrev[:], AF.Exp, bias=negmax[:, 0:1])
                nc.vector.tensor_tensor(mrev[:], m_row[:], mrev[:], OP.mult)
                csum = spool.tile([1, 1], F32, name=f"csum{pfx}{b}", tag="csum", bufs=1)
                nc.vector.tensor_reduce(csum[:], mrev[:], axis=mybir.AxisListType.X, op=OP.add)
                crecip = spool.tile([1, 1], F32, name=f"crec{pfx}{b}", tag="crec", bufs=1)
                nc.vector.reciprocal(crecip[:], csum[:])
                c2r_sb = spool.tile([1, L], BF16, name=f"c2r{pfx}{b}", tag="c2r", bufs=1)
                nc.vector.tensor_scalar(c2r_sb[:], mrev[:], crecip[:, 0:1], None, op0=OP.mult)
                nc.sync.dma_start(out=c2rd[(b, pfx)].ap()[0:L], in_=c2r_sb[:])
                nc.sync.dma_start(out=c2rd[(b, pfx)].ap()[L:], in_=c2r_sb[:])

                # ---- aggregation
                cf = apool.tile([128, NT * L], BF16, name=f"cf{pfx}{b}", tag="big32", bufs=1)
                nc.sync.dma_start(
                    out=cf[:].rearrange("p (j l) -> p j l", j=NT),
                    in_=bass.AP(c2rd[(b, pfx)], 0, [[1, 128], [128, NT], [1, L]]))
                agg_t = apool.tile([128, NCH * L], F32R, name=f"agg{pfx}{b}", tag="q", bufs=1)
                for m in range(NCH):
                    for h in range(2):
                        pt = ps_tile(f"pag{m}{h}")
                        for j in range(NT):
                            nc.tensor.matmul(
                                pt[:], vrev_t[:, j * E + 128 * m : j * E + 128 * m + 128],
                                cf[:, j * L + 512 * h : j * L + 512 * h + 512],
                                start=(j == 0), stop=(j == NT - 1))
                        nc.vector.tensor_copy(
                            agg_t[:, m * L + 512 * h : m * L + 512 * h + 512], pt[:])
                proj_el(out_y_t, agg_t, w_in[f"{pfx}_wo"], BO[pfx],
                        resid=resid, resid_dram=resid_dram)

            # ============== main program (stage-major across batches) ==============
            xel_t, xrev_t, y_t, x_t = {}, {}, {}, {}
            for b in range(BPC):
                xel = apool.tile([128, NCH * L], F32R, name=f"xel{b}", tag="xel", bufs=1)
                nc.sync.dma_start(out=xel[:].rearrange("p (c l) -> p c l", c=NCH),
                                  in_=xt_in.ap()[b].rearrange("(c p) l -> p c l", p=128))
                xrev = apool.tile([128, NCH * L], F32R, name=f"xrev{b}", tag="xrev", bufs=1)
                nc.sync.dma_start(out=xrev[:].rearrange("p (c l) -> p c l", c=NCH),
                                  in_=xtr_in.ap()[b].rearrange("(c p) l -> p c l", p=128))
                xel_t[b], xrev_t[b] = xel, xrev
            for b in range(BPC):
                y1 = apool.tile([128, NCH * L], F32, name=f"y1{b}", tag="y", bufs=2)
                attention(b, "sa", xel_t[b], xel_t[b], xrev_t[b], y1,
                          resid_dram=xt_in.ap()[b])
                y_t[b] = y1
            for b in range(BPC):
                x2 = apool.tile([128, NCH * L], F32R, name=f"x2{b}", tag="x", bufs=2)
                decomp(b, y_t[b], x2, True)
                x_t[b] = x2
            for b in range(BPC):
                ent = apool.tile([128, NCH * L], F32R, name=f"ent{b}", tag="xel", bufs=1)
                nc.sync.dma_start(out=ent[:].rearrange("p (c l) -> p c l", c=NCH),
                                  in_=ent_in.ap()[b].rearrange("(c p) l -> p c l", p=128))
                entr = apool.tile([128, NCH * L], F32R, name=f"entr{b}", tag="xrev", bufs=1)
                nc.sync.dma_start(out=entr[:].rearrange("p (c l) -> p c l", c=NCH),
                                  in_=entr_in.ap()[b].rearrange("(c p) l -> p c l", p=128))
                xel_t[b], xrev_t[b] = ent, entr
            for b in range(BPC):
                y2 = apool.tile([128, NCH * L], F32, name=f"y2{b}", tag="y", bufs=2)
                attention(b, "ca", x_t[b], xel_t[b], xrev_t[b], y2, resid=x_t[b])
                y_t[b] = y2
            for b in range(BPC):
                x3 = apool.tile([128, NCH * L], F32R, name=f"x3{b}", tag="x", bufs=2)
                decomp(b, y_t[b], x3, False)
                x_t[b] = x3
            fcw = apool.tile([128, 16384], BF16, name="fcw", tag="big32", bufs=1)
            nc.sync.dma_start(out=fcw[:], in_=fcw_in.ap())
            for b in range(BPC):
                x3 = x_t[b]
                x3bf = apool.tile([128, NCH * L], BF16, name=f"x3bf{b}", tag="xrev", bufs=1)
                nc.vector.tensor_copy(x3bf[:], x3[:])
                y3 = apool.tile([128, NCH * L], F32, name=f"y3{b}", tag="y", bufs=2)
                for half in range(2):
                    h_t = apool.tile([128, NXP * 512], BF16, name=f"h{b}{half}", tag="v", bufs=1)
                    for xc in range(NXP):
                        pt = ps_tile(f"ph{xc}")
                        for c in range(NCH):
                            nc.tensor.matmul(
                                pt[:], fcw[:, c * 2048 + 128 * xc : c * 2048 + 128 * xc + 128],
                                x3bf[:, c * L + 512 * half : c * L + 512 * half + 512],
                                start=(c == 0), stop=(c == NCH - 1))
                        nc.scalar.activation(h_t[:, xc * 512 : (xc + 1) * 512], pt[:],
                                             AF.Gelu, bias=bias_pc[:, FC1B + xc : FC1B + xc + 1])
                    for m in range(NCH):
                        pt = ps_tile(f"pf{m}")
                        for xc in range(NXP):
                            nc.tensor.matmul(
                                pt[:],
                                fcw[:, 8192 + xc * 512 + 128 * m : 8192 + xc * 512 + 128 * m + 128],
                                h_t[:, xc * 512 : (xc + 1) * 512],
                                start=(xc == 0), stop=(xc == NXP - 1))
                        sl = slice(m * L + 512 * half, m * L + 512 * half + 512)
                        nc.vector.scalar_tensor_tensor(
                            y3[:, sl], pt[:], bias_pc[:, FC2B + m : FC2B + m + 1],
                            x3[:, sl], op0=OP.add, op1=OP.add)
                y_t[b] = y3
            for b in range(BPC):
                x4 = apool.tile([128, NCH * L], F32R, name=f"x4{b}", tag="x", bufs=2)
                decomp(b, y_t[b], x4, False)
                x_t[b] = x4
            for b in range(BPC):
                x4 = x_t[b]
                sq = apool.tile([128, NCH * L], F32R, name=f"sq{b}", tag="q", bufs=1)
                for c in range(NCH):
                    nc.scalar.activation(sq[:, c * L : (c + 1) * L],
                                         x4[:, c * L : (c + 1) * L], AF.Square)
                mu_ps = pmpool.tile([1, L], F32, name=f"mups{b}", tag="psbig")
                for h in range(2):
                    for c in range(NCH):
                        nc.tensor.matmul(mu_ps[0:1, 512 * h : 512 * h + 512], ones_div[:],
                                         x4[:, c * L + 512 * h : c * L + 512 * h + 512],
                                         start=(c == 0), stop=(c == NCH - 1))
                mu_r = spool.tile([1, L], F32, name=f"mur{b}", tag="mrow", bufs=1)
                nc.vector.tensor_copy(mu_r[:], mu_ps[:])
                ms_ps = pmpool.tile([1, L], F32, name=f"msps{b}", tag="psbig")
                for h in range(2):
                    for c in range(NCH):
                        nc.tensor.matmul(ms_ps[0:1, 512 * h : 512 * h + 512], ones_div[:],
                                         sq[:, c * L + 512 * h : c * L + 512 * h + 512],
                                         start=(c == 0), stop=(c == NCH - 1))
                var_r = spool.tile([1, L], F32, name=f"varr{b}", tag="mrev", bufs=1)
                nc.vector.tensor_tensor(var_r[:], mu_r[:], mu_r[:], OP.mult)
                nc.vector.scalar_tensor_tensor(var_r[:], ms_ps[:], 1e-5, var_r[:],
                                               op0=OP.add, op1=OP.subtract)
                nc.scalar.activation(var_r[:], var_r[:], AF.Sqrt)
                rows = spool.tile([1, L], F32R, name=f"rows{b}", tag="c2r", bufs=1)
                rows2 = spool.tile([1, L], F32R, name=f"rows2{b}", tag="rows2", bufs=1)
                with nc.allow_low_precision(reason="istd broadcast is f32r by design"):
                    nc.vector.reciprocal(rows[:], var_r[:])
                nc.vector.tensor_tensor(rows2[:], mu_r[:], rows[:], OP.mult)
                bc = apool.tile([128, 2 * L], F32, name=f"bc{b}", tag="xrev", bufs=1)
                for h in range(4):
                    bp = ps_tile(f"bc{h}")
                    src_row = rows if h < 2 else rows2
                    nc.tensor.matmul(bp[:], ones_row[:],
                                     src_row[0:1, 512 * (h % 2) : 512 * (h % 2) + 512],
                                     start=True, stop=True)
                    nc.vector.tensor_copy(bc[:, 512 * h : 512 * h + 512], bp[:])
                seas = apool.tile([128, NCH * L], F32, name=f"seas{b}", tag="q", bufs=1)
                accs = spool.tile([128, NCH], F32, name=f"accs{b}", tag="accs", bufs=1)
                for c in range(NCH):
                    t1 = spool.tile([128, L], F32, name=f"t1{c}", tag="ws", bufs=2)
                    nc.vector.tensor_tensor(t1[:], x4[:, c * L : (c + 1) * L],
                                            bc[:, 0:L], OP.mult)
                    nc.vector.tensor_tensor(t1[:], t1[:], bc[:, L:], OP.subtract)
                    nc.scalar.activation(seas[:, c * L : (c + 1) * L], t1[:], AF.Identity,
                                         bias=bias_pc[:, LNB + c : LNB + c + 1],
                                         scale=bias_pc[:, LNG + c : LNG + c + 1],
                                         accum_out=accs[:, c : c + 1])
                for c in range(NCH):
                    nc.vector.tensor_scalar(accs[:, c : c + 1], accs[:, c : c + 1],
                                            1.0 / L, None, op0=OP.mult)
                    nc.vector.tensor_scalar(seas[:, c * L : (c + 1) * L],
                                            seas[:, c * L : (c + 1) * L],
                                            accs[:, c : c + 1], None, op0=OP.subtract)
                for a in range(NT):
                    tp = ps_tile(f"tps{a}")
                    for c in range(NCH):
                        nc.tensor.transpose(tp[:, 128 * c : 128 * c + 128],
                                            seas[:, c * L + 128 * a : c * L + 128 * a + 128],
                                            ident[:, :])
                    osb = spool.tile([128, 512], F32, name=f"osb{a}", tag="osb", bufs=2)
                    nc.vector.tensor_copy(osb[:], tp[:])
                    nc.sync.dma_start(out=seas_out.ap()[b, 128 * a : 128 * a + 128, :],
                                      in_=osb[:])
            wct = apool.tile([128, 12 * F], F32R, name="wctt", tag="big32", bufs=1)
            nc.sync.dma_start(out=wct[:].rearrange("p (c n) -> p c n", c=12),
                              in_=wct_in.ap().rearrange("(c p) n -> p c n", p=128))
            for b in range(BPC):
                tpad = apool.tile([128, NCH * 1026], F32R, name=f"tpad{b}", tag="y", bufs=2)
                nc.sync.dma_start(
                    out=tpad[:].rearrange("p (c l) -> p c l", c=NCH)[:, :, 1:1025],
                    in_=trd[b].ap().bitcast(F32R).rearrange("c p l -> p c l"))
                for c in range(NCH):
                    nc.vector.tensor_copy(tpad[:, c * 1026 : c * 1026 + 1],
                                          tpad[:, c * 1026 + L : c * 1026 + L + 1])
                    nc.vector.tensor_copy(tpad[:, c * 1026 + 1025 : c * 1026 + 1026],
                                          tpad[:, c * 1026 + 1 : c * 1026 + 2])
                for a in range(NT):
                    pt = ps_tile(f"ptc{a}")
                    n = 0
                    for j in range(3):
                        for c in range(NCH):
                            nc.tensor.matmul(
                                pt[:],
                                tpad[:, c * 1026 + 128 * a + j : c * 1026 + 128 * a + j + 128],
                                wct[:, (j * NCH + c) * F : (j * NCH + c) * F + F],
                                start=(n == 0), stop=(n == 11))
                            n += 1
                    osb = spool.tile([128, 512], F32, name=f"osc{a}", tag="osb", bufs=2)
                    nc.vector.tensor_copy(osb[:], pt[:])
                    nc.sync.dma_start(out=trend_out.ap()[b, 128 * a : 128 * a + 128, :],
                                      in_=osb[:])

    nc.compile()
    return nc


def _host_prep(inputs):
    f32 = np.float32
    x = np.asarray(inputs["x"], f32)
    enc = np.asarray(inputs["enc_output"], f32)
    xt = np.ascontiguousarray(x.transpose(0, 2, 1))
    xtr = np.ascontiguousarray(xt[:, :, ::-1])
    ent = np.ascontiguousarray(enc.transpose(0, 2, 1))
    entr = np.ascontiguousarray(ent[:, :, ::-1])

    shared = {}
    for p in ("sa", "ca"):
        for nme in ("wq", "wk", "wv", "wo"):
            shared[f"{p}_{nme}"] = np.ascontiguousarray(np.asarray(inputs[f"{p}_{nme}"], f32))
    import ml_dtypes
    fc1 = np.asarray(inputs["fc1_w"], f32).reshape(NCH, 128, XP)       # (c, p, xp)
    fc2 = np.asarray(inputs["fc2_w"], f32).reshape(NXP, 128, E)        # (xc, p, e)
    fcw = np.zeros((128, 16384), ml_dtypes.bfloat16)
    fcw[:, :8192] = fc1.transpose(1, 0, 2).reshape(128, 8192).astype(ml_dtypes.bfloat16)
    fcw[:, 8192:] = fc2.transpose(1, 0, 2).reshape(128, 8192).astype(ml_dtypes.bfloat16)
    shared["fcw"] = fcw
    tw = np.asarray(inputs["trend_w"], f32)
    shared["wct"] = np.ascontiguousarray(tw.transpose(2, 1, 0).reshape(3 * E, F))

    def pc(v, nch=4):
        return np.ascontiguousarray(np.asarray(v, f32).reshape(nch, 128).T)

    shared["bias_pc"] = np.ascontiguousarray(np.concatenate([
        pc(inputs["sa_bq"]), pc(inputs["sa_bk"]), pc(inputs["sa_bo"]),
        pc(inputs["ca_bq"]), pc(inputs["ca_bk"]), pc(inputs["ca_bo"]),
        pc(inputs["fc2_b"]), pc(inputs["ln_g"]), pc(inputs["ln_b"]),
        pc(inputs["fc1_b"], 16),
    ], axis=1))
    shared["brow"] = np.ascontiguousarray(np.concatenate(
        [np.asarray(inputs["sa_bv"], f32), np.asarray(inputs["ca_bv"], f32)])[None, :])
    shared["ident"] = np.eye(128, dtype=f32)
    shared["identr"] = np.eye(128, dtype=f32)
    shared["jmat"] = np.ascontiguousarray(np.eye(128, dtype=f32)[::-1])
    shared["ones_div"] = np.full((128, 1), 1.0 / E, f32)
    shared["ones_row"] = np.ones((1, 128), f32)
    ramp = np.zeros(16, f32)
    ramp[:13] = np.arange(12, -1, -1)
    shared["rampl"] = np.tile(ramp, (128, 1))
    shared["ones12"] = np.ones((128, 12), f32)

    in_maps = []
    for core in range(NCORES):
        s = slice(core * BPC, (core + 1) * BPC)
        m = dict(shared)
        m["xt"] = np.ascontiguousarray(xt[s])
        m["xtr"] = np.ascontiguousarray(xtr[s])
        m["ent"] = np.ascontiguousarray(ent[s])
        m["entr"] = np.ascontiguousarray(entr[s])
        in_maps.append(m)
    return in_maps


_LAST = {}


def kernel(**inputs):
    from concourse.bass_utils import run_bass_kernel_spmd

    nc = _build()
    in_maps = _host_prep(inputs)
    res = run_bass_kernel_spmd(nc, in_maps, core_ids=list(range(NCORES)),
                               **_LAST.get("kwargs", {}))
    _LAST["res"] = res
    seasonal = np.concatenate([res.results[c]["seasonal"] for c in range(NCORES)], axis=0)
    trend = np.concatenate([res.results[c]["trend"] for c in range(NCORES)], axis=0)
    return seasonal, trend


# revision 21
# speedup vs baseline: 1.1334x; 1.0161x over previous
"""Trainium2 Bass kernel for nn_DecoderBlock (Autoformer decoder layer).

Data-parallel over batch: 16 batches -> 8 cores x 2 batches, per-batch
serial pipeline on each core. Layout is channel-major (E on partitions).

Autocorrelation without FFTs:
  M[tau] = (1/E) sum_t <k_t, q_{(t+tau)%L}>
via 128-row tiles of K^T Q whose wrapped diagonals are summed by a DRAM
shear round-trip (row stride 1153 on readback) + ones-matmul reduction.
Top-6 + softmax give a scattered weight vector; the roll-aggregation
  agg[e,t] = sum_{s'} Vrev[s',e] * c2R[s'+t]
uses host-time-reversed V inputs and the reversed weight vector written
twice to DRAM, so all access patterns are positive-stride.

Precision: score matmuls in true fp32 (top-6 margins ~1e-4 in M units);
FFN in bf16 (its output is ~0.2x the residual scale, so bf16 noise
dilutes well below tolerance); everything else float32r.
"""
import functools
import numpy as np

NCORES = 8
BPC = 2
L = 1024
E = 512
XP = 2048
F = 512
KS = 25
NCH = 4
NT = 8
NXP = 16
SCORES_F32R = True


@functools.lru_cache(maxsize=1)
def _build():
    import concourse.bacc as bacc
    import concourse.bass as bass
    import concourse.mybir as mybir
    from concourse.tile import TileContext

    F32 = mybir.dt.float32
    F32R = mybir.dt.float32r
    BF16 = mybir.dt.bfloat16
    AF = mybir.ActivationFunctionType
    OP = mybir.AluOpType

    SCORE_DT = F32R if SCORES_F32R else F32
    nc = bacc.Bacc("TRN2", target_bir_lowering=False, debug=False, num_devices=NCORES)

    def din(name, shape, dtype=F32R):
        return nc.declare_dram_parameter(name, list(shape), dtype, isOutput=False)

    xt_in = din("xt", (BPC, E, L))
    xtr_in = din("xtr", (BPC, E, L))
    ent_in = din("ent", (BPC, E, L))
    entr_in = din("entr", (BPC, E, L))
    w_in = {}
    for p in ("sa", "ca"):
        for nme in ("wq", "wk", "wv", "wo"):
            w_in[f"{p}_{nme}"] = din(f"{p}_{nme}", (E, E))
    fcw_in = din("fcw", (128, 16384), BF16)   # packed fc1(c,2048)|fc2(xc,512), bf16
    wct_in = din("wct", (3 * E, F))
    bias_pc_in = din("bias_pc", (128, 52), F32)
    brow_in = din("brow", (1, 2 * E), F32R)
    ident_in = din("ident", (128, 128), F32)
    identr_in = din("identr", (128, 128), F32R)
    jmat_in = din("jmat", (128, 128), F32)
    ones_div_in = din("ones_div", (128, 1))
    ones_row_in = din("ones_row", (1, 128))
    rampl_in = din("rampl", (128, 16), F32)
    ones12_in = din("ones12", (128, 12), F32)

    seas_out = nc.declare_dram_parameter("seasonal", [BPC, L, E], F32, isOutput=True)

    trend_out = nc.declare_dram_parameter("trend", [BPC, L, F], F32, isOutput=True)

    a2d = {(b, p): nc.dram_tensor(f"a2d_{p}{b}", [L, 1152], F32R)
           for b in range(BPC) for p in ("sa", "ca")}
    c2rd = {(b, p): nc.dram_tensor(f"c2rd_{p}{b}", [2 * L], BF16)
            for b in range(BPC) for p in ("sa", "ca")}
    trd = {b: nc.dram_tensor(f"trdram{b}", [NCH, 128, L], F32) for b in range(BPC)}

    BQ = {"sa": 0, "ca": 12}
    BK = {"sa": 4, "ca": 16}
    BO = {"sa": 8, "ca": 20}
    FC2B, LNG, LNB, FC1B = 24, 28, 32, 36
    BVROW = {"sa": 0, "ca": E}

    with TileContext(nc) as tc:
        with (
            tc.tile_pool(name="consts", bufs=1) as cpool,
            tc.tile_pool(name="wst", bufs=4) as wstp,
            tc.tile_pool(name="acts", bufs=1) as apool,
            tc.tile_pool(name="str", bufs=1) as spool,
            tc.tile_pool(name="pp", bufs=2, space="PSUM") as pspool,
            tc.tile_pool(name="pa", bufs=1, space="PSUM") as papool,
            tc.tile_pool(name="pm", bufs=1, space="PSUM") as pmpool,
        ):
            ident = cpool.tile([128, 128], F32, name="ident")
            identr = cpool.tile([128, 128], F32R, name="identr")
            jmat = cpool.tile([128, 128], F32, name="jmat")
            ones_div = cpool.tile([128, 1], F32R, name="ones_div")
            ones_row = cpool.tile([1, 128], F32R, name="ones_row")
            rampl = cpool.tile([128, 16], F32, name="rampl")
            ones12 = cpool.tile([128, 12], F32, name="ones12")
            bias_pc = cpool.tile([128, 52], F32, name="bias_pc")
            brow = cpool.tile([1, 2 * E], F32R, name="brow")
            for t, d in ((ident, ident_in), (identr, identr_in), (jmat, jmat_in),
                         (ones_div, ones_div_in), (ones_row, ones_row_in),
                         (rampl, rampl_in), (ones12, ones12_in),
                         (bias_pc, bias_pc_in), (brow, brow_in)):
                nc.sync.dma_start(out=t[:], in_=d.ap())

            def ps_tile(name):
                return pspool.tile([128, 512], F32, name=name, tag="pp")

            def wchunk(dram, sl0, sl1, name):
                """stream a (128, 512) f32r weight chunk: rows sl0 (c-range), cols sl1."""
                t = wstp.tile([128, 512], F32R, name=name, tag="wst")
                nc.sync.dma_start(
                    out=t[:].rearrange("p (c n) -> p c n", c=(sl0.stop - sl0.start))
                    if False else t[:],
                    in_=dram.ap().rearrange("(c p) n -> p c n", p=128)[:, sl0, sl1]
                    .rearrange("p c n -> p (c n)"))
                return t

            def proj_el(out_t, src_t, w_dram, bcol, resid=None, resid_dram=None):
                """out[e_out,t] = sum_e w[e,e_out] src[e,t] + bias (+resid)."""
                for m in range(NCH):
                    wm = wstp.tile([128, 512], F32R, name=f"wm{m}", tag="wst")
                    nc.sync.dma_start(
                        out=wm[:].rearrange("p (c n) -> p c n", c=NCH),
                        in_=w_dram.ap().rearrange("(c p) n -> p c n", p=128)
                            [:, :, 128 * m : 128 * m + 128])
                    for h in range(2):
                        pt = ps_tile(f"pp{m}{h}")
                        first = True
                        if resid is not None:
                            nc.tensor.matmul(pt[:], identr[:],
                                             resid[:, m * L + 512 * h : m * L + 512 * h + 512],
                                             start=True, stop=False)
                            first = False
                        elif resid_dram is not None:
                            rs = wstp.tile([128, 512], F32R, name=f"rs{m}{h}", tag="wst")
                            nc.sync.dma_start(
                                out=rs[:],
                                in_=resid_dram.rearrange("(c p) l -> p c l", p=128)
                                    [:, m, 512 * h : 512 * h + 512])
                            nc.tensor.matmul(pt[:], identr[:], rs[:], start=True, stop=False)
                            first = False
                        for c in range(NCH):
                            nc.tensor.matmul(
                                pt[:], wm[:, c * 128 : c * 128 + 128],
                                src_t[:, c * L + 512 * h : c * L + 512 * h + 512],
                                start=(first and c == 0), stop=(c == NCH - 1))
                        dst = out_t[:, m * L + 512 * h : m * L + 512 * h + 512]
                        if (m + h) % 2 == 0:
                            nc.vector.tensor_scalar(dst, pt[:],
                                bias_pc[:, bcol + m : bcol + m + 1], None, op0=OP.add)
                        else:
                            nc.scalar.activation(dst, pt[:], AF.Identity,
                                bias=bias_pc[:, bcol + m : bcol + m + 1])

            def proj_rev(out_t, src_rev_t, w_dram, bvcol):
                """time-reversed V in (L,E): out[s',e] = sum_e' xrev[e',s'] w[e',e] + bv."""
                wv = []
                for c in range(NCH):
                    wc = wstp.tile([128, 512], F32R, name=f"wvc{c}", tag="wst")
                    nc.sync.dma_start(
                        out=wc[:],
                        in_=w_dram.ap().rearrange("(c p) n -> p c n", p=128)[:, c, :])
                    wv.append(wc)
                for j in range(NT):
                    pt = ps_tile(f"pv{j}")
                    nc.tensor.matmul(pt[:], ones_row[:], brow[0:1, bvcol : bvcol + E],
                                     start=True, stop=False)
                    for c in range(NCH):
                        nc.tensor.matmul(
                            pt[:], src_rev_t[:, c * L + 128 * j : c * L + 128 * j + 128],
                            wv[c][:], start=False, stop=(c == NCH - 1))
                    if j % 2 == 0:
                        nc.vector.tensor_copy(out_t[:, j * E : (j + 1) * E], pt[:])
                    else:
                        nc.scalar.activation(out_t[:, j * E : (j + 1) * E], pt[:], AF.Copy)

            def decomp(b, y_t, xnext_t, first_tr):
                """xnext = y - movavg25(y); DRAM trend (+)= movavg25(y)."""
                for c in range(NCH):
                    y = lambda a, bb: y_t[:, c * L + a : c * L + bb]
                    ws = spool.tile([128, L], F32, name=f"ws{c}", tag="ws", bufs=2)
                    cs25 = spool.tile([128, 25], F32, name=f"cs25{c}", tag="cs25", bufs=1)
                    nc.vector.tensor_tensor_scan(cs25[:], y(0, 25), y(0, 25), 0.0,
                                                 op0=OP.add, op1=OP.bypass)
                    nc.vector.scalar_tensor_tensor(ws[:, 0:13], rampl[:, 0:13], y(0, 1),
                                                   cs25[:, 12:25], op0=OP.mult, op1=OP.add)
                    nc.vector.tensor_tensor_scan(ws[:, 13:1012], y(25, L), y(0, 999),
                                                 cs25[:, 24:25], op0=OP.add, op1=OP.subtract)
                    ylast = spool.tile([128, 12], F32, name=f"yl{c}", tag="yl", bufs=2)
                    nc.vector.tensor_scalar(ylast[:], ones12[:], y(L - 1, L), None, op0=OP.mult)
                    nc.vector.tensor_tensor_scan(ws[:, 1012:1024], ylast[:], y(999, 1011),
                                                 ws[:, 1011:1012], op0=OP.add, op1=OP.subtract)
                    nc.vector.scalar_tensor_tensor(
                        xnext_t[:, c * L : (c + 1) * L], ws[:], -1.0 / KS, y(0, L),
                        op0=OP.mult, op1=OP.add)
                    # trend accumulation in DRAM: ws *= 1/25 then (accum-)DMA out
                    nc.vector.tensor_scalar(ws[:], ws[:], 1.0 / KS, None, op0=OP.mult)
                    if first_tr:
                        nc.gpsimd.dma_start(out=trd[b].ap()[c], in_=ws[:])
                    else:
                        nc.gpsimd.dma_start(out=trd[b].ap()[c], in_=ws[:],
                                            accum_op=OP.add)

            def attention(b, pfx, xq_t, kv_t, kvrev_t, out_y_t, resid=None, resid_dram=None):
                q_t = apool.tile([128, NCH * L], SCORE_DT, name=f"q{pfx}{b}", tag="q", bufs=1)
                k_t = apool.tile([128, NCH * L], SCORE_DT, name=f"k{pfx}{b}", tag="big32", bufs=1)
                proj_el(q_t, xq_t, w_in[f"{pfx}_wq"], BQ[pfx])
                proj_el(k_t, kv_t, w_in[f"{pfx}_wk"], BK[pfx])

                # ---- scores (fp32)
                a2 = a2d[(b, pfx)]
                m_ps = pmpool.tile([1, L], F32, name=f"mps{pfx}{b}", tag="psbig")
                for i in range(NT):
                    # pa banks: h=0 at 0, h=1 at 512, h=2 at 1024; the (at most
                    # one) wrapped split piece goes to its own bank at 1536 so
                    # every accumulation group has exactly one start=True owner.
                    pa = papool.tile([128, 2048], F32, name=f"pa{i}", tag="pa")
                    split_h, split_n1 = -1, 0
                    for c in range(NCH):
                        lhs = k_t[:, c * L + 128 * i : c * L + 128 * i + 128]
                        for h in range(3):
                            w0 = 128 * i + 384 * h
                            st, sp = (c == 0), (c == NCH - 1)
                            o = 512 * h
                            if w0 + 384 <= L:
                                nc.tensor.matmul(pa[:, o : o + 384], lhs,
                                                 q_t[:, c * L + w0 : c * L + w0 + 384],
                                                 start=st, stop=sp)
                            elif w0 >= L:
                                nc.tensor.matmul(pa[:, o : o + 384], lhs,
                                                 q_t[:, c * L + w0 - L : c * L + w0 - L + 384],
                                                 start=st, stop=sp)
                            else:
                                n1 = L - w0
                                split_h, split_n1 = h, n1
                                nc.tensor.matmul(pa[:, o : o + n1], lhs,
                                                 q_t[:, c * L + w0 : c * L + L],
                                                 start=st, stop=sp)
                                nc.tensor.matmul(pa[:, 1536 : 1536 + 384 - n1], lhs,
                                                 q_t[:, c * L : c * L + 384 - n1],
                                                 start=st, stop=sp)
                    a2sb = spool.tile([128, 1152], F32R, name=f"a2sb{i}", tag="a2sb", bufs=1)
                    for h in range(3):
                        if h == split_h:
                            n1 = split_n1
                            nc.vector.tensor_copy(a2sb[:, 384 * h : 384 * h + n1],
                                                  pa[:, 512 * h : 512 * h + n1])
                            nc.vector.tensor_copy(a2sb[:, 384 * h + n1 : 384 * h + 384],
                                                  pa[:, 1536 : 1536 + 384 - n1])
                        else:
                            nc.vector.tensor_copy(a2sb[:, 384 * h : 384 * h + 384],
                                                  pa[:, 512 * h : 512 * h + 384])
                    nc.sync.dma_start(out=a2.ap()[128 * i : 128 * i + 128, :], in_=a2sb[:])
                    sch = spool.tile([128, L], F32R, name=f"sch{i}", tag="ws", bufs=2)
                    nc.sync.dma_start(
                        out=sch[:], in_=bass.AP(a2, 1152 * 128 * i, [[1153, 128], [1, L]]))

                    for h in range(2):
                        nc.tensor.matmul(m_ps[0:1, 512 * h : 512 * h + 512], ones_div[:],
                                         sch[:, 512 * h : 512 * h + 512],
                                         start=(i == 0), stop=(i == NT - 1))

                # ---- V projection (after scores so "v" slot turns over late)
                vrev_t = apool.tile([128, NT * E], BF16, name=f"v{pfx}{b}", tag="v", bufs=1)
                proj_rev(vrev_t, kvrev_t, w_in[f"{pfx}_wv"], BVROW[pfx])

                # ---- reverse M; scattered softmax -> c2R
                m_row = spool.tile([1, L], F32, name=f"mrow{pfx}{b}", tag="mrow", bufs=1)
                nc.vector.tensor_copy(m_row[:], m_ps[:])
                colt_ps = pspool.tile([128, 8], F32, name=f"colt{pfx}{b}", tag="pp")
                for g in range(8):
                    nc.tensor.transpose(colt_ps[:, g : g + 1],
                                        m_row[0:1, 128 * g : 128 * g + 128], ident[0:1, 0:1])
                colt = spool.tile([128, 8], F32, name=f"coltsb{pfx}{b}", tag="coltsb", bufs=1)
                nc.vector.tensor_copy(colt[:], colt_ps[:])
                revt_ps = pspool.tile([128, 8], F32, name=f"revt{pfx}{b}", tag="pp")
                nc.tensor.matmul(revt_ps[:], jmat[:], colt[:], start=True, stop=True)
                revt = spool.tile([128, 8], F32, name=f"revtsb{pfx}{b}", tag="revtsb", bufs=1)
                nc.vector.tensor_copy(revt[:], revt_ps[:])
                mrev_ps = pmpool.tile([1, L], F32, name=f"mrevps{pfx}{b}", tag="psbig")
                for g in range(8):
                    nc.tensor.transpose(mrev_ps[0:1, 128 * (7 - g) : 128 * (7 - g) + 128],
                                        revt[:, g : g + 1], ident[:, :])
                mrev = spool.tile([1, L], F32, name=f"mrev{pfx}{b}", tag="mrev", bufs=1)
                nc.vector.tensor_copy(mrev[:], mrev_ps[:])

                top8 = spool.tile([1, 8], F32, name=f"top8{pfx}{b}", tag="top8", bufs=1)
                nc.vector.max(top8[:], mrev[:])
                negmax = spool.tile([1, 1], F32, name=f"ngm{pfx}{b}", tag="ngm", bufs=1)
                nc.vector.tensor_scalar(negmax[:], top8[:, 0:1], -1.0, None, op0=OP.mult)
                # mask overwrites m_row (dead); exp result then masked in place
                nc.vector.tensor_scalar(m_row[:], mrev[:], top8[:, 5:6], None, op0=OP.is_ge)
                nc.scalar.activation(mrev[:], mrev[:], AF.Exp, bias=negmax[:, 0:1])
                nc.vector.tensor_tensor(mrev[:], m_row[:], mrev[:], OP.mult)
                csum = spool.tile([1, 1], F32, name=f"csum{pfx}{b}", tag="csum", bufs=1)
                nc.vector.tensor_reduce(csum[:], mrev[:], axis=mybir.AxisListType.X, op=OP.add)
                crecip = spool.tile([1, 1], F32, name=f"crec{pfx}{b}", tag="crec", bufs=1)
                nc.vector.reciprocal(crecip[:], csum[:])
                c2r_sb = spool.tile([1, L], BF16, name=f"c2r{pfx}{b}", tag="c2r", bufs=1)
                nc.vector.tensor_scalar(c2r_sb[:], mrev[:], crecip[:, 0:1], None, op0=OP.mult)
                nc.sync.dma_start(out=c2rd[(b, pfx)].ap()[0:L], in_=c2r_sb[:])
                nc.sync.dma_start(out=c2rd[(b, pfx)].ap()[L:], in_=c2r_sb[:])

                # ---- aggregation
                cf = apool.tile([128, NT * L], BF16, name=f"cf{pfx}{b}", tag="big32", bufs=1)
                nc.sync.dma_start(
                    out=cf[:].rearrange("p (j l) -> p j l", j=NT),
                    in_=bass.AP(c2rd[(b, pfx)], 0, [[1, 128], [128, NT], [1, L]]))
                agg_t = apool.tile([128, NCH * L], F32R, name=f"agg{pfx}{b}", tag="q", bufs=1)
                for m in range(NCH):
                    for h in range(2):
                        pt = ps_tile(f"pag{m}{h}")
                        for j in range(NT):
                            nc.tensor.matmul(
                                pt[:], vrev_t[:, j * E + 128 * m : j * E + 128 * m + 128],
                                cf[:, j * L + 512 * h : j * L + 512 * h + 512],
                                start=(j == 0), stop=(j == NT - 1))
                        dsta = agg_t[:, m * L + 512 * h : m * L + 512 * h + 512]
                        if (m + h) % 2 == 0:
                            nc.vector.tensor_copy(dsta, pt[:])
                        else:
                            nc.scalar.activation(dsta, pt[:], AF.Copy)
                proj_el(out_y_t, agg_t, w_in[f"{pfx}_wo"], BO[pfx],
                        resid=resid, resid_dram=resid_dram)

            # ============== main program (stage-major across batches) ==============
            xel_t, xrev_t, y_t, x_t = {}, {}, {}, {}
            for b in range(BPC):
                xel = apool.tile([128, NCH * L], F32R, name=f"xel{b}", tag="xel", bufs=1)
                nc.sync.dma_start(out=xel[:].rearrange("p (c l) -> p c l", c=NCH),
                                  in_=xt_in.ap()[b].rearrange("(c p) l -> p c l", p=128))
                xrev = apool.tile([128, NCH * L], F32R, name=f"xrev{b}", tag="xrev", bufs=1)
                nc.sync.dma_start(out=xrev[:].rearrange("p (c l) -> p c l", c=NCH),
                                  in_=xtr_in.ap()[b].rearrange("(c p) l -> p c l", p=128))
                xel_t[b], xrev_t[b] = xel, xrev
            for b in range(BPC):
                y1 = apool.tile([128, NCH * L], F32, name=f"y1{b}", tag="y", bufs=2)
                attention(b, "sa", xel_t[b], xel_t[b], xrev_t[b], y1,
                          resid_dram=xt_in.ap()[b])
                y_t[b] = y1
            for b in range(BPC):
                x2 = apool.tile([128, NCH * L], F32R, name=f"x2{b}", tag="x", bufs=2)
                decomp(b, y_t[b], x2, True)
                x_t[b] = x2
            for b in range(BPC):
                ent = apool.tile([128, NCH * L], F32R, name=f"ent{b}", tag="xel", bufs=1)
                nc.sync.dma_start(out=ent[:].rearrange("p (c l) -> p c l", c=NCH),
                                  in_=ent_in.ap()[b].rearrange("(c p) l -> p c l", p=128))
                entr = apool.tile([128, NCH * L], F32R, name=f"entr{b}", tag="xrev", bufs=1)
                nc.sync.dma_start(out=entr[:].rearrange("p (c l) -> p c l", c=NCH),
                                  in_=entr_in.ap()[b].rearrange("(c p) l -> p c l", p=128))
                xel_t[b], xrev_t[b] = ent, entr
            for b in range(BPC):
                y2 = apool.tile([128, NCH * L], F32, name=f"y2{b}", tag="y", bufs=2)
                attention(b, "ca", x_t[b], xel_t[b], xrev_t[b], y2, resid=x_t[b])
                y_t[b] = y2
            for b in range(BPC):
                x3 = apool.tile([128, NCH * L], F32R, name=f"x3{b}", tag="x", bufs=2)
                decomp(b, y_t[b], x3, False)
                x_t[b] = x3
            fcw = apool.tile([128, 16384], BF16, name="fcw", tag="big32", bufs=1)
            nc.sync.dma_start(out=fcw[:], in_=fcw_in.ap())
            for b in range(BPC):
                x3 = x_t[b]
                x3bf = apool.tile([128, NCH * L], BF16, name=f"x3bf{b}", tag="xrev", bufs=1)
                nc.vector.tensor_copy(x3bf[:], x3[:])
                y3 = apool.tile([128, NCH * L], F32, name=f"y3{b}", tag="y", bufs=2)
                for half in range(2):
                    h_t = apool.tile([128, NXP * 512], BF16, name=f"h{b}{half}", tag="v", bufs=1)
                    for xc in range(NXP):
                        pt = ps_tile(f"ph{xc}")
                        for c in range(NCH):
                            nc.tensor.matmul(
                                pt[:], fcw[:, c * 2048 + 128 * xc : c * 2048 + 128 * xc + 128],
                                x3bf[:, c * L + 512 * half : c * L + 512 * half + 512],
                                start=(c == 0), stop=(c == NCH - 1))
                        nc.scalar.activation(h_t[:, xc * 512 : (xc + 1) * 512], pt[:],
                                             AF.Gelu, bias=bias_pc[:, FC1B + xc : FC1B + xc + 1])
                    for m in range(NCH):
                        pt = ps_tile(f"pf{m}")
                        for xc in range(NXP):
                            nc.tensor.matmul(
                                pt[:],
                                fcw[:, 8192 + xc * 512 + 128 * m : 8192 + xc * 512 + 128 * m + 128],
                                h_t[:, xc * 512 : (xc + 1) * 512],
                                start=(xc == 0), stop=(xc == NXP - 1))
                        sl = slice(m * L + 512 * half, m * L + 512 * half + 512)
                        nc.vector.scalar_tensor_tensor(
                            y3[:, sl], pt[:], bias_pc[:, FC2B + m : FC2B + m + 1],
                            x3[:, sl], op0=OP.add, op1=OP.add)
                y_t[b] = y3
            for b in range(BPC):
                x4 = apool.tile([128, NCH * L], F32R, name=f"x4{b}", tag="x", bufs=2)
                decomp(b, y_t[b], x4, False)
                x_t[b] = x4
            for b in range(BPC):
                x4 = x_t[b]
                sq = apool.tile([128, NCH * L], F32R, name=f"sq{b}", tag="q", bufs=1)
                for c in range(NCH):
                    nc.scalar.activation(sq[:, c * L : (c + 1) * L],
                                         x4[:, c * L : (c + 1) * L], AF.Square)
                mu_ps = pmpool.tile([1, L], F32, name=f"mups{b}", tag="psbig")
                for h in range(2):
                    for c in range(NCH):
                        nc.tensor.matmul(mu_ps[0:1, 512 * h : 512 * h + 512], ones_div[:],
                                         x4[:, c * L + 512 * h : c * L + 512 * h + 512],
                                         start=(c == 0), stop=(c == NCH - 1))
                mu_r = spool.tile([1, L], F32, name=f"mur{b}", tag="mrow", bufs=1)
                nc.vector.tensor_copy(mu_r[:], mu_ps[:])
                ms_ps = pmpool.tile([1, L], F32, name=f"msps{b}", tag="psbig")
                for h in range(2):
                    for c in range(NCH):
                        nc.tensor.matmul(ms_ps[0:1, 512 * h : 512 * h + 512], ones_div[:],
                                         sq[:, c * L + 512 * h : c * L + 512 * h + 512],
                                         start=(c == 0), stop=(c == NCH - 1))
                var_r = spool.tile([1, L], F32, name=f"varr{b}", tag="mrev", bufs=1)
                nc.vector.tensor_tensor(var_r[:], mu_r[:], mu_r[:], OP.mult)
                nc.vector.scalar_tensor_tensor(var_r[:], ms_ps[:], 1e-5, var_r[:],
                                               op0=OP.add, op1=OP.subtract)
                nc.scalar.activation(var_r[:], var_r[:], AF.Sqrt)
                rows = spool.tile([1, L], F32R, name=f"rows{b}", tag="c2r", bufs=1)
                rows2 = spool.tile([1, L], F32R, name=f"rows2{b}", tag="rows2", bufs=1)
                with nc.allow_low_precision(reason="istd broadcast is f32r by design"):
                    nc.vector.reciprocal(rows[:], var_r[:])
                nc.vector.tensor_tensor(rows2[:], mu_r[:], rows[:], OP.mult)
                bc = apool.tile([128, 2 * L], F32, name=f"bc{b}", tag="xrev", bufs=1)
                for h in range(4):
                    bp = ps_tile(f"bc{h}")
                    src_row = rows if h < 2 else rows2
                    nc.tensor.matmul(bp[:], ones_row[:],
                                     src_row[0:1, 512 * (h % 2) : 512 * (h % 2) + 512],
                                     start=True, stop=True)
                    nc.vector.tensor_copy(bc[:, 512 * h : 512 * h + 512], bp[:])
                seas = apool.tile([128, NCH * L], F32, name=f"seas{b}", tag="q", bufs=1)
                accs = spool.tile([128, NCH], F32, name=f"accs{b}", tag="accs", bufs=1)
                for c in range(NCH):
                    t1 = spool.tile([128, L], F32, name=f"t1{c}", tag="ws", bufs=2)
                    nc.vector.tensor_tensor(t1[:], x4[:, c * L : (c + 1) * L],
                                            bc[:, 0:L], OP.mult)
                    nc.vector.tensor_tensor(t1[:], t1[:], bc[:, L:], OP.subtract)
                    nc.scalar.activation(seas[:, c * L : (c + 1) * L], t1[:], AF.Identity,
                                         bias=bias_pc[:, LNB + c : LNB + c + 1],
                                         scale=bias_pc[:, LNG + c : LNG + c + 1],
                                         accum_out=accs[:, c : c + 1])
                for c in range(NCH):
                    nc.vector.tensor_scalar(accs[:, c : c + 1], accs[:, c : c + 1],
                                            1.0 / L, None, op0=OP.mult)
                    nc.vector.tensor_scalar(seas[:, c * L : (c + 1) * L],
                                            seas[:, c * L : (c + 1) * L],
                                            accs[:, c : c + 1], None, op0=OP.subtract)
                for a in range(NT):
                    tp = ps_tile(f"tps{a}")
                    for c in range(NCH):
                        nc.tensor.transpose(tp[:, 128 * c : 128 * c + 128],
                                            seas[:, c * L + 128 * a : c * L + 128 * a + 128],
                                            ident[:, :])
                    osb = spool.tile([128, 512], F32, name=f"osb{a}", tag="osb", bufs=2)
                    nc.vector.tensor_copy(osb[:], tp[:])
                    nc.sync.dma_start(out=seas_out.ap()[b, 128 * a : 128 * a + 128, :],
                                      in_=osb[:])
            wct = apool.tile([128, 12 * F], F32R, name="wctt", tag="big32", bufs=1)
            nc.sync.dma_start(out=wct[:].rearrange("p (c n) -> p c n", c=12),
                              in_=wct_in.ap().rearrange("(c p) n -> p c n", p=128))
            for b in range(BPC):
                tpad = apool.tile([128, NCH * 1026], F32R, name=f"tpad{b}", tag="y", bufs=2)
                nc.sync.dma_start(
                    out=tpad[:].rearrange("p (c l) -> p c l", c=NCH)[:, :, 1:1025],
                    in_=trd[b].ap().bitcast(F32R).rearrange("c p l -> p c l"))
                for c in range(NCH):
                    nc.vector.tensor_copy(tpad[:, c * 1026 : c * 1026 + 1],
                                          tpad[:, c * 1026 + L : c * 1026 + L + 1])
                    nc.vector.tensor_copy(tpad[:, c * 1026 + 1025 : c * 1026 + 1026],
                                          tpad[:, c * 1026 + 1 : c * 1026 + 2])
                for a in range(NT):
                    pt = ps_tile(f"ptc{a}")
                    n = 0
                    for j in range(3):
                        for c in range(NCH):
                            nc.tensor.matmul(
                                pt[:],
                                tpad[:, c * 1026 + 128 * a + j : c * 1026 + 128 * a + j + 128],
                                wct[:, (j * NCH + c) * F : (j * NCH + c) * F + F],
                                start=(n == 0), stop=(n == 11))
                            n += 1
                    osb = spool.tile([128, 512], F32, name=f"osc{a}", tag="osb", bufs=2)
                    nc.vector.tensor_copy(osb[:], pt[:])
                    nc.sync.dma_start(out=trend_out.ap()[b, 128 * a : 128 * a + 128, :],
                                      in_=osb[:])

    nc.compile()
    return nc


def _host_prep(inputs):
    f32 = np.float32
    x = np.asarray(inputs["x"], f32)
    enc = np.asarray(inputs["enc_output"], f32)
    xt = np.ascontiguousarray(x.transpose(0, 2, 1))
    xtr = np.ascontiguousarray(xt[:, :, ::-1])
    ent = np.ascontiguousarray(enc.transpose(0, 2, 1))
    entr = np.ascontiguousarray(ent[:, :, ::-1])

    shared = {}
    for p in ("sa", "ca"):
        for nme in ("wq", "wk", "wv", "wo"):
            shared[f"{p}_{nme}"] = np.ascontiguousarray(np.asarray(inputs[f"{p}_{nme}"], f32))
    import ml_dtypes
    fc1 = np.asarray(inputs["fc1_w"], f32).reshape(NCH, 128, XP)       # (c, p, xp)
    fc2 = np.asarray(inputs["fc2_w"], f32).reshape(NXP, 128, E)        # (xc, p, e)
    fcw = np.zeros((128, 16384), ml_dtypes.bfloat16)
    fcw[:, :8192] = fc1.transpose(1, 0, 2).reshape(128, 8192).astype(ml_dtypes.bfloat16)
    fcw[:, 8192:] = fc2.transpose(1, 0, 2).reshape(128, 8192).astype(ml_dtypes.bfloat16)
    shared["fcw"] = fcw
    tw = np.asarray(inputs["trend_w"], f32)
    shared["wct"] = np.ascontiguousarray(tw.transpose(2, 1, 0).reshape(3 * E, F))

    def pc(v, nch=4):
        return np.ascontiguousarray(np.asarray(v, f32).reshape(nch, 128).T)

    shared["bias_pc"] = np.ascontiguousarray(np.concatenate([
        pc(inputs["sa_bq"]), pc(inputs["sa_bk"]), pc(inputs["sa_bo"]),
        pc(inputs["ca_bq"]), pc(inputs["ca_bk"]), pc(inputs["ca_bo"]),
        pc(inputs["fc2_b"]), pc(inputs["ln_g"]), pc(inputs["ln_b"]),
        pc(inputs["fc1_b"], 16),
    ], axis=1))
    shared["brow"] = np.ascontiguousarray(np.concatenate(
        [np.asarray(inputs["sa_bv"], f32), np.asarray(inputs["ca_bv"], f32)])[None, :])
    shared["ident"] = np.eye(128, dtype=f32)
    shared["identr"] = np.eye(128, dtype=f32)
    shared["jmat"] = np.ascontiguousarray(np.eye(128, dtype=f32)[::-1])
    shared["ones_div"] = np.full((128, 1), 1.0 / E, f32)
    shared["ones_row"] = np.ones((1, 128), f32)
    ramp = np.zeros(16, f32)
    ramp[:13] = np.arange(12, -1, -1)
    shared["rampl"] = np.tile(ramp, (128, 1))
    shared["ones12"] = np.ones((128, 12), f32)

    in_maps = []
    for core in range(NCORES):
        s = slice(core * BPC, (core + 1) * BPC)
        m = dict(shared)
        m["xt"] = np.ascontiguousarray(xt[s])
        m["xtr"] = np.ascontiguousarray(xtr[s])
        m["ent"] = np.ascontiguousarray(ent[s])
        m["entr"] = np.ascontiguousarray(entr[s])
        in_maps.append(m)
    return in_maps


_LAST = {}


def kernel(**inputs):
    from concourse.bass_utils import run_bass_kernel_spmd

    nc = _build()
    in_maps = _host_prep(inputs)
    res = run_bass_kernel_spmd(nc, in_maps, core_ids=list(range(NCORES)),
                               **_LAST.get("kwargs", {}))
    _LAST["res"] = res
    seasonal = np.concatenate([res.results[c]["seasonal"] for c in range(NCORES)], axis=0)
    trend = np.concatenate([res.results[c]["trend"] for c in range(NCORES)], axis=0)
    return seasonal, trend
